# revision 1
# baseline (speedup 1.0000x reference)
"""Causal multi-head attention (8 heads, 1x1-conv projections) on 8 TRN2 cores.

Sharding: data-parallel over batch N=8 -> one batch element per NeuronCore.
Per-core kernel (S=1024 pixels, C=E=256 channels, H=8 heads, d=32):
  q = WqT.T @ x, k = WkT.T @ x              (e, s) layout, fp32r matmuls
  vT = x.T @ WvT                            (s, e) layout (transposed v, so the
                                            attention contraction needs no
                                            on-chip transpose of big tensors)
  per head: P^T[sk, sq] = exp(mask(k_h^T q_h))   scores computed TRANSPOSED so
                                            softmax denominator comes from an
                                            appended ones-column in v (M=33)
  out_h = (vAug_h^T @ P^T) -> rows 0..31 numerator^T, row 32 = denominator
  att = num / denom (per-column broadcast via gpsimd partition_broadcast)
  out = WprojT.T @ att + bproj_eff
Host folds: 1/sqrt(d) into Wq/bq; v-bias through the projection
(bproj_eff = bproj + Wproj @ bv, valid because attention rows sum to 1).
P^T and v^T are bf16 (fp32 accumulate); projections and scores are fp32r.

Scheduling notes (vs the first working version):
- Input DMAs are consolidated (packed wq||wk tensor split by m-chunk, packed
  bias vector, host-provided diag-mask tile) and ordered by the critical
  path: wqk[m=0], x[j=0], biases, x[j=1], wqk[m=1], mask, wv, wp.  The ones
  column of vAug and the ones row for the PE broadcast are memset on-chip.
- The diagonal-block causal mask is a bf16 elementwise multiply on DVE
  (mask tile), not a gpsimd affine_select: keeps Pool off the per-chunk
  critical chain.
- The softmax denominator broadcast uses gpsimd partition_broadcast
  (attn ucode library) -- no DRAM bounce, no HWDGE traffic.
- Head 0's first four chunks exp per 512-column window so the ACT exp
  stream starts as soon as the first x half lands.
- The m=0 output projection fires as soon as heads 0..3 are normalized
  (during head 4), and the tail splits head 7's second attn@v half into two
  256-column sub-chains with a DMA-free PE-broadcast normalization, so the
  post-last-exp serial chain is short.
- f32r score matmuls below 256 output columns run at 1/4 rate; the two
  short windows per head are widened to 256 columns (extra columns land in
  PSUM but are never read by exp).
"""

import numpy as np

N_CORES = 8
C = 256      # input channels
E = 256      # embed channels (q/k)
O = 256      # v/out channels
S = 1024     # spatial positions (32*32)
H = 8        # heads
D = 32       # head dim
NCH = 2      # 256 = 2 * 128 partition chunks

_CACHE = {}


def _build_program():
    import concourse.mybir as mybir
    from concourse import bacc
    from concourse import library_config
    from concourse.tile import TileContext

    F32 = mybir.dt.float32
    F32R = mybir.dt.float32r
    BF16 = mybir.dt.bfloat16
    EXP = mybir.ActivationFunctionType.Exp

    nc = bacc.Bacc("TRN2", target_bir_lowering=False, debug=False)

    # fp32r inputs: DMA is an accepted f32r producer, PE rounds on read
    xin = nc.dram_tensor("xin", [C, S], F32R, kind="ExternalInput")
    wqk0 = nc.dram_tensor("wqk0", [C, 2 * 128], F32R, kind="ExternalInput")
    wqk1 = nc.dram_tensor("wqk1", [C, 2 * 128], F32R, kind="ExternalInput")
    wvt = nc.dram_tensor("wvt", [C, O], F32R, kind="ExternalInput")
    wpt = nc.dram_tensor("wpt", [O, O], F32R, kind="ExternalInput")
    biasd = nc.dram_tensor("biasd", [3 * 256], F32, kind="ExternalInput")
    maskd = nc.dram_tensor("maskd", [128, 128], BF16, kind="ExternalInput")
    outd = nc.dram_tensor("out", [O, S], F32, kind="ExternalOutput")

    with TileContext(nc) as tc:
        with (
            tc.tile_pool(name="cst", bufs=1) as cst,
            tc.tile_pool(name="ptp", bufs=3) as ptp,
            tc.tile_pool(name="rbp", bufs=4) as rbp,
            tc.tile_pool(name="osb", bufs=2) as osb,
            tc.tile_pool(name="psc", bufs=2, space="PSUM") as psc,
            tc.tile_pool(name="pav", bufs=4, space="PSUM") as pav,
        ):
            # gpsimd ucode: partition_broadcast lives in the attn library
            nc.gpsimd.load_library(library_config.attn)

            # --- preload exp table + warm the PE clock while DMAs run ---
            dmz = cst.tile([128, 64], F32, tag="dmz")
            nc.vector.memset(dmz, 0.0)
            dme = cst.tile([128, 1], F32, tag="dme")
            nc.scalar.activation(dme, dmz[:, 0:1], EXP)
            wup = cst.tile([128, 64], F32R, tag="wup")
            nc.vector.tensor_copy(wup, dmz)
            pwu = pav.tile([64, 512], F32, tag="pa")
            for _ in range(36):
                nc.tensor.matmul(pwu[:, 0:64], wup, wup[:, 0:64].bitcast(F32R),
                                 start=True, stop=True)

            # on-chip constants (no DMA): vAug ones column + PE-broadcast row
            vaug = cst.tile([128, 8, H, D + 1], BF16, tag="vaug")
            nc.vector.memset(vaug[:, :, :, D], 1.0)
            onc = cst.tile([33, 32], F32, tag="onc")
            nc.vector.memset(onc[32:33, :], 1.0)

            # --- input DMAs ordered along the critical path ---
            wqk = cst.tile([128, NCH, NCH, 2, 128], F32R, tag="wqk")
            wqk_src = [
                d.ap().rearrange("(c p) (t e) -> p c t e", p=128, t=2)
                for d in (wqk0, wqk1)
            ]
            xr = cst.tile([128, NCH, S], F32R, tag="xr")
            xsrc = xin.ap().rearrange("(c p) s -> p c s", p=128)
            bt = cst.tile([128, 3, NCH], F32, tag="bt")
            maskt = cst.tile([128, 128], BF16, tag="maskt")
            wv = cst.tile([128, NCH, 256], F32R, tag="wv")
            wp = cst.tile([128, NCH, 256], F32R, tag="wp")

            nc.sync.dma_start(out=wqk[:, 0], in_=wqk_src[0])
            nc.sync.dma_start(out=xr[:, :, 0:512], in_=xsrc[:, :, 0:512])
            nc.sync.dma_start(
                out=bt, in_=biasd.ap().rearrange("(b m p) -> p b m", p=128, b=3)
            )
            nc.sync.dma_start(out=xr[:, :, 512:1024], in_=xsrc[:, :, 512:1024])
            nc.sync.dma_start(out=wqk[:, 1], in_=wqk_src[1])
            nc.sync.dma_start(out=maskt, in_=maskd.ap())
            nc.sync.dma_start(out=wv, in_=wvt.ap().rearrange("(c p) e -> p c e", p=128))
            nc.sync.dma_start(out=wp, in_=wpt.ap().rearrange("(c p) e -> p c e", p=128))

            q_sb = cst.tile([128, NCH, S], F32R, tag="q_sb")
            k_sb = cst.tile([128, NCH, S], F32R, tag="k_sb")
            att = cst.tile([128, NCH, S], F32R, tag="att")

            def qk_proj_unit(t, m, j, pool=None):
                # t: 0 = q, 1 = k
                dst = (q_sb, k_sb)[t]
                pp = (pool or psc).tile([128, 512], F32, tag="sc" if pool is None else "pa")
                for c in range(2):
                    nc.tensor.matmul(
                        pp,
                        wqk[:, m, c, t, :],
                        xr[:, c, j * 512:(j + 1) * 512],
                        start=(c == 0), stop=(c == 1),
                    )
                if t == 1 and m == 0 and j == 0:
                    nc.scalar.add(
                        dst[:, m, j * 512:(j + 1) * 512], pp, bt[:, t, m:m + 1]
                    )
                else:
                    nc.vector.tensor_scalar_add(
                        dst[:, m, j * 512:(j + 1) * 512], pp, bt[:, t, m:m + 1]
                    )

            def v_proj_unit(i):
                pv = psc.tile([128, 512], F32, tag="sc")
                for c in range(2):
                    nc.tensor.matmul(
                        pv[:, 0:256],
                        xr[:, c, i * 128:(i + 1) * 128],
                        wv[:, c, :],
                        start=(c == 0), stop=(c == 1),
                    )
                nc.vector.tensor_copy(
                    vaug[:, i, :, 0:D],
                    pv[:, 0:256].rearrange("p (h d) -> p h d", h=H),
                )

            def mask_mult(pts, i):
                # zero the strictly-lower part of the diagonal block
                nc.vector.tensor_mul(
                    pts[:, i, 128 * i:128 * (i + 1)],
                    pts[:, i, 128 * i:128 * (i + 1)],
                    maskt,
                )

            def scores_win(h, ps, pts, i, j):
                # matmul + exp for sq window j (512 cols) of sk chunk i
                m, r = h // 4, h % 4
                rows = slice(32 * r, 32 * r + 32)
                we = 512 * (j + 1)
                if we <= 128 * i:
                    return
                ws = max(512 * j, 128 * i)
                # f32r matmuls under 256 output columns run at 1/4 rate:
                # widen (exp never reads the extra columns)
                ws_mm = max(min(ws, we - 256), 512 * j)
                nc.tensor.matmul(
                    ps[:, ws_mm:we],
                    k_sb[rows, m, 128 * i:128 * (i + 1)],
                    q_sb[rows, m, ws_mm:we],
                    start=True, stop=True,
                    tile_position=(32 * r, 0),
                )
                nc.scalar.activation(pts[:, i, ws:we], ps[:, ws:we], EXP)
                if ws <= 128 * i:
                    mask_mult(pts, i)

            def scores_chunk(h, pts, i):
                # whole-chunk variant: window matmuls, then one exp.  Chunks
                # >= 4 only touch columns 512:1024, so they take a 1-bank
                # psum slot from pav -- keeps the 2-slot psc ring free of
                # head-boundary write-after-read stalls.
                m, r = h // 4, h % 4
                rows = slice(32 * r, 32 * r + 32)
                if i < 4:
                    ps = psc.tile([128, S], F32, tag="sc")
                    off = 0
                else:
                    ps = pav.tile([128, 512], F32, tag="pa")
                    off = 512
                for j in range(2):
                    we = 512 * (j + 1)
                    if we <= 128 * i:
                        continue
                    ws = max(512 * j, 128 * i)
                    ws_mm = max(min(ws, we - 256), 512 * j)
                    nc.tensor.matmul(
                        ps[:, ws_mm - off:we - off],
                        k_sb[rows, m, 128 * i:128 * (i + 1)],
                        q_sb[rows, m, ws_mm:we],
                        start=True, stop=True,
                        tile_position=(32 * r, 0),
                    )
                nc.scalar.activation(pts[:, i, 128 * i:S], ps[:, 128 * i - off:S - off], EXP)
                mask_mult(pts, i)

            def attnv_mms(h, pts, j, pa, ii, first, last_mm):
                # a group of attn@v accumulation matmuls for sq-half j.
                # These read only completed prior-head data, so the schedule
                # sprinkles them between score matmuls as PE filler instead
                # of one long block that would starve the exp stream.
                for idx, i in enumerate(ii):
                    ws = max(512 * j, 128 * i)
                    we = 512 * (j + 1)
                    nc.tensor.matmul(
                        pa[:, ws - 512 * j:we - 512 * j],
                        vaug[:, i, h, :],
                        pts[:, i, ws:we],
                        start=(first and idx == 0),
                        stop=(last_mm and idx == len(ii) - 1),
                    )

            def attnv_bcast(h, j, pa):
                # reciprocal + partition broadcast; per-column multiply is
                # deferred (attnv_mul) so it never sits in front of later
                # masks in the DVE queue while waiting on the Pool broadcast
                m, r = h // 4, h % 4
                rf = rbp.tile([1, 512], F32, tag="rf")
                nc.vector.reciprocal(rf, pa[32:33, :])
                rb = rbp.tile([32, 512], F32, tag="rb")
                nc.gpsimd.partition_broadcast(rb, rf)
                return (pa, rb, m, r, j)

            def attnv_mul(state):
                pa, rb, m, r, j = state
                nc.vector.tensor_mul(
                    att[32 * r:32 * r + 32, m, 512 * j:512 * (j + 1)],
                    pa[0:32, :], rb,
                )

            def attnv_tail_mms(h, pts, q0, q1, pa, ii, first, last_mm):
                # partial attn@v accumulation for sq columns [q0, q1)
                for idx, i in enumerate(ii):
                    ws = max(q0, 128 * i)
                    nc.tensor.matmul(
                        pa[:, ws - q0:q1 - q0],
                        vaug[:, i, h, :],
                        pts[:, i, ws:q1],
                        start=(first and idx == 0),
                        stop=(last_mm and idx == len(ii) - 1),
                    )

            def attnv_tail_norm(h, pa, q0, q1):
                # PE-broadcast normalization (DMA- and Pool-free tail chain)
                m, r = h // 4, h % 4
                w = q1 - q0
                rf = rbp.tile([33, 512], F32R, tag="rff")
                with nc.allow_low_precision(reason="softmax recip in f32r"):
                    nc.vector.reciprocal(rf[:, 0:w], pa[:, 0:w])
                pb = pav.tile([32, 512], F32, tag="pa")
                nc.tensor.matmul(pb[:, 0:w], onc[32:33, :].bitcast(F32R), rf[32:33, 0:w],
                                 start=True, stop=True)
                rb = rbp.tile([32, 512], F32, tag="rb")
                nc.scalar.copy(rb[:, 0:w], pb[:, 0:w])
                nc.vector.tensor_mul(
                    att[32 * r:32 * r + 32, m, q0:q1],
                    pa[0:32, 0:w], rb[:, 0:w],
                )

            out_ap = outd.ap().rearrange("(m p) s -> p m s", p=128)

            def outproj_unit(m, q0, q1, ot2=None):
                # output projection + bias for out-chunk m, columns [q0, q1)
                w = q1 - q0
                po = pav.tile([128, 512], F32, tag="pa")
                for c in range(2):
                    nc.tensor.matmul(
                        po[:, 0:w],
                        wp[:, c, m * 128:(m + 1) * 128],
                        att[:, c, q0:q1],
                        start=(c == 0), stop=(c == 1),
                    )
                if ot2 is not None:
                    nc.vector.tensor_scalar_add(
                        ot2[:, m, 0:w], po[:, 0:w], bt[:, 2, m:m + 1])
                    return
                ot = osb.tile([128, 512], F32, tag="ot")
                nc.scalar.add(ot[:, 0:w], po[:, 0:w], bt[:, 2, m:m + 1])
                nc.sync.dma_start(out=out_ap[:, m, q0:q1], in_=ot[:, 0:w])

            def outproj_pair(q0, q1):
                # both m chunks into one shared tile -> single output DMA
                w = q1 - q0
                ot2 = osb.tile([128, 2, 512], F32, tag="ot2")
                outproj_unit(0, q0, q1, ot2=ot2)
                outproj_unit(1, q0, q1, ot2=ot2)
                nc.sync.dma_start(out=out_ap[:, :, q0:q1], in_=ot2[:, :, 0:w])

            # --- emission schedule ---
            pts_tiles = {}

            def get_pts(h):
                if h not in pts_tiles:
                    pts = ptp.tile([128, 8, S], BF16, tag="pts")
                    pts_tiles[h] = pts
                return pts_tiles[h]

            # head 0 primes the ACT stream: j=0 windows of chunks 0..3 only
            # need the first x half.  Two [128, S] psum slots ping-pong
            # across the four chunks; the j=1 q/k projections borrow pav
            # psum so they don't alias the in-flight score windows.
            qk_proj_unit(0, 0, 0)
            qk_proj_unit(1, 0, 0)
            pts0 = get_pts(0)
            slot_a = psc.tile([128, S], F32, tag="sc")
            slot_b = psc.tile([128, S], F32, tag="sc")
            ps0 = {0: slot_a, 1: slot_b, 2: slot_a, 3: slot_b}
            for i in range(4):
                scores_win(0, ps0[i], pts0, i, 0)
            qk_proj_unit(0, 0, 1, pool=pav)
            qk_proj_unit(1, 0, 1, pool=pav)
            for i in range(4):
                scores_win(0, ps0[i], pts0, i, 1)

            def sc(h, i):
                scores_chunk(h, get_pts(h), i)

            st0 = st1 = st70 = None
            pa0 = pa1 = pp7 = None
            for h in range(H):
                last = h == H - 1
                prev = pts_tiles.get(h - 1)
                for i in range(1, 8):
                    if not (h == 0 and i < 4):
                        if i == 7 and not last:
                            sc(h + 1, 0)
                        sc(h, i)
                    # prior-head attn@v accumulation, group-wise PE filler.
                    # Each half's reciprocal + Pool broadcast launches well
                    # before its deferred multiply, so the multiply never
                    # stalls the DVE queue (masks feed the PE from there).
                    if h > 0:
                        if i == 1:
                            pa0 = pav.tile([33, 512], F32, tag="pa")
                            attnv_mms(h - 1, prev, 0, pa0, [0, 1, 2, 3], True, True)
                            st0 = attnv_bcast(h - 1, 0, pa0)
                        if i == 3:
                            pa1 = pav.tile([33, 512], F32, tag="pa")
                            attnv_mms(h - 1, prev, 1, pa1, [0, 1, 2, 3], True, False)
                            attnv_mul(st0)
                        if i == 4:
                            attnv_mms(h - 1, prev, 1, pa1, [4, 5, 6, 7], False, True)
                        if i == 5:
                            st1 = attnv_bcast(h - 1, 1, pa1)
                            pts_tiles.pop(h - 1)
                        if i == 6:
                            attnv_mul(st1)
                    if last:
                        if i == 4:
                            pp7 = pav.tile([33, 512], F32, tag="pa")
                            attnv_mms(h, pts_tiles[h], 0, pp7, [0, 1, 2, 3], True, True)
                            st70 = attnv_bcast(h, 0, pp7)
                        if i == 5:
                            attnv_mul(st70)
                    if h == 0 and 1 <= i <= 4:
                        v_proj_unit(2 * (i - 1))
                        v_proj_unit(2 * (i - 1) + 1)
                    if h == 1 and 1 <= i <= 4:
                        qk_proj_unit((i + 1) % 2, 1, (i - 1) // 2)
                # keep the exp stream fed across the head boundary: emit
                # nothing else here; attnv of this head is deferred
            # tail: head 7.  attnv(7, 0) was emitted at i == 5, so att's
            # j=0 half completes for all heads while the last exps retire;
            # the j=1 half is split into two 256-column sub-chains, with the
            # [768:1024) accumulation pre-issued for chunks 0..6 so only one
            # 128-column matmul plus a short PE-broadcast normalization and
            # a 256-column projection remain after the final exp.
            p7 = pts_tiles[H - 1]
            pa_a = pav.tile([33, 512], F32, tag="pa")
            attnv_tail_mms(H - 1, p7, 512, 768, pa_a,
                           list(range(6)), True, True)
            pa_b = pav.tile([33, 512], F32, tag="pa")
            attnv_tail_mms(H - 1, p7, 768, 1024, pa_b,
                           list(range(7)), True, False)
            attnv_tail_norm(H - 1, pa_a, 512, 768)
            attnv_tail_mms(H - 1, p7, 768, 1024, pa_b, [7], False, True)
            attnv_tail_norm(H - 1, pa_b, 768, 1024)
            outproj_unit(0, 0, 512)
            outproj_unit(1, 0, 512)
            outproj_pair(512, 768)
            outproj_pair(768, 1024)

    nc.compile()
    return nc


def get_program():
    if "nc" not in _CACHE:
        _CACHE["nc"] = _build_program()
    return _CACHE["nc"]


def kernel(x, wq, bq, wkv, bkv, wproj, bproj):
    import ml_dtypes
    from concourse.bass_utils import run_bass_kernel_spmd

    nc = get_program()

    x = np.asarray(x, dtype=np.float32)
    n = x.shape[0]
    assert n == N_CORES and x.shape[1:] == (C, 32, 32)

    scale = 1.0 / np.sqrt(np.float32(D))
    wq_s = np.asarray(wq, np.float32) * scale
    bq_s = np.asarray(bq, np.float32) * scale
    wk = np.asarray(wkv[:E], np.float32)
    bk = np.asarray(bkv[:E], np.float32)
    wv = np.asarray(wkv[E:], np.float32)
    bv = np.asarray(bkv[E:], np.float32)
    wproj = np.asarray(wproj, np.float32)
    bproj_eff = (np.asarray(bproj, np.float32)
                 + wproj.astype(np.float64) @ bv.astype(np.float64)).astype(np.float32)

    # mask[sk, sq] keeps sq >= sk (upper triangle incl. diagonal of the
    # transposed scores' diagonal block)
    mask = np.triu(np.ones((128, 128), np.float32)).astype(ml_dtypes.bfloat16)

    shared = {
        "wqk0": np.ascontiguousarray(
            np.concatenate([wq_s.T[:, 0:128], wk.T[:, 0:128]], axis=1)),
        "wqk1": np.ascontiguousarray(
            np.concatenate([wq_s.T[:, 128:256], wk.T[:, 128:256]], axis=1)),
        "wvt": np.ascontiguousarray(wv.T),
        "wpt": np.ascontiguousarray(wproj.T),
        "biasd": np.ascontiguousarray(
            np.concatenate([bq_s, bk, bproj_eff])),
        "maskd": mask,
    }
    in_maps = [
        {"xin": np.ascontiguousarray(x[i].reshape(C, S)), **shared}
        for i in range(N_CORES)
    ]
    res = run_bass_kernel_spmd(nc, in_maps, core_ids=list(range(N_CORES)))
    out = np.stack([res.results[i]["out"].reshape(O, 32, 32) for i in range(N_CORES)])
    return out.astype(np.float32)



# revision 41
# speedup vs baseline: 1.1080x; 1.1080x over previous
"""Causal multi-head attention (8 heads, 1x1-conv projections) on 8 TRN2 cores.

Sharding: data-parallel over batch N=8 -> one batch element per NeuronCore.
Per-core kernel (S=1024 pixels, C=E=256 channels, H=8 heads, d=32):
  q = WqT.T @ x, k = WkT.T @ x              (e, s) layout, fp32r matmuls
  vT = x.T @ WvT                            (s, e) layout
  per head: P^T[sk, sq] = exp(mask(k_h^T q_h))   scores computed TRANSPOSED so
                                            softmax denominator comes from an
                                            appended ones-column in v (M=33)
  out_h = (vAug_h^T @ P^T) -> rows 0..31 numerator^T, row 32 = denominator
  att = num / denom (bf16), out = WprojT.T @ att + bproj_eff

Design (v2, ACT-stream-paced):
- The exp stream on the Activation engine is the pacing resource (~44us of
  exp work).  Everything else is scheduled around keeping it gap-free.
- Causal masking is folded into the score matmuls: an extra bf16 accumulate
  matmul (identity stationary x strictly-lower -BIG moving) writes -1e30
  into the masked diagonal-block region of PSUM before the exp, so exp
  emits exact zeros and the DVE never touches masks.
- Per head h: attnv j0-half accumulates during head h (after exp chunk 3),
  the j1-half during head h+1.  Softmax normalization: DVE reciprocal of
  the denominator row, gpsimd partition_broadcast, deferred DVE multiply.
- exp emission order per head: e1..e5, e0(next head), e6, e7 -- psum-ring
  write-after-read hazards then never block a score matmul near its exp.
- PSUM: 2x[128,1024] ring (chunks 0-3 + outproj), 2x[128,512] ring
  (chunks 4-7 + q/k/v projections), 2x[33,512] ring (attnv accumulators).
- Output projection + DMA run in 4 column pieces ([0:256], [256:512],
  [512:768], [768:1024]) pipelined against the last head's exp tail; att
  and Wproj are bf16 so narrow matmuls run at full PE rate.
- Startup: x[:, 0:512] lands early (one DMA queue, transfers serialize),
  PE warmup matmuls start at ~250ns so the PE clock is ramped before the
  first projection; head 0 primes the stream with j0-window exps.
"""

import numpy as np

N_CORES = 8
C = 256      # input channels
E = 256      # embed channels (q/k)
O = 256      # v/out channels
S = 1024     # spatial positions (32*32)
H = 8        # heads
D = 32       # head dim
NCH = 2      # 256 = 2 * 128 partition chunks
N_WARM = 33  # PE clock-warm matmuls

_CACHE = {}


def _build_program():
    import concourse.mybir as mybir
    from concourse import bacc
    from concourse import library_config
    from concourse.tile import TileContext

    F32 = mybir.dt.float32
    F32R = mybir.dt.float32r
    BF16 = mybir.dt.bfloat16
    EXP = mybir.ActivationFunctionType.Exp

    nc = bacc.Bacc("TRN2", target_bir_lowering=False, debug=False)

    xin = nc.dram_tensor("xin", [C, S], F32R, kind="ExternalInput")
    wqk0 = nc.dram_tensor("wqk0", [C, 2 * 128], F32R, kind="ExternalInput")
    wqk1 = nc.dram_tensor("wqk1", [C, 2 * 128], F32R, kind="ExternalInput")
    wvt = nc.dram_tensor("wvt", [C, O], F32R, kind="ExternalInput")
    wpt = nc.dram_tensor("wpt", [O, O], BF16, kind="ExternalInput")
    biasd = nc.dram_tensor("biasd", [3 * 256], F32, kind="ExternalInput")
    bprowd = nc.dram_tensor("bprow", [1, 256], BF16, kind="ExternalInput")
    # msk2[:, 0:128] = identity, msk2[:, 128:256] = -1e30 * strict_lower
    msk2d = nc.dram_tensor("msk2", [128, 256], BF16, kind="ExternalInput")
    outd = nc.dram_tensor("out", [O, S], F32, kind="ExternalOutput")

    with TileContext(nc) as tc:
        with (
            tc.tile_pool(name="cst", bufs=1) as cst,
            tc.tile_pool(name="ptp", bufs=3) as ptp,
            tc.tile_pool(name="rbp", bufs=4) as rbp,
            tc.tile_pool(name="osb", bufs=4) as osb,
            tc.tile_pool(name="big", bufs=2, space="PSUM") as big,
            tc.tile_pool(name="sml", bufs=2, space="PSUM") as sml,
            tc.tile_pool(name="pap", bufs=2, space="PSUM") as pap,
        ):
            # --- fast start: Pool memsets, ACT table preload, PE clock warm
            wup = cst.tile([128, 64], F32, tag="wup")
            nc.gpsimd.memset(wup, 0.0)
            dmz = cst.tile([128, 1], F32, tag="dmz")
            nc.gpsimd.memset(dmz, 0.0)
            nc.gpsimd.load_library(library_config.attn)
            dme = cst.tile([128, 1], F32, tag="dme")
            nc.scalar.activation(dme, dmz, EXP)
            pwu = big.tile([64, 512], F32, tag="bg")
            for _ in range(N_WARM):
                nc.tensor.matmul(pwu[:, 0:64], wup.bitcast(F32R),
                                 wup[:, 0:64].bitcast(F32R),
                                 start=True, stop=True)

            vaug = cst.tile([128, 8, H, D + 1], BF16, tag="vaug")
            nc.vector.memset(vaug[:, :, :, D], 1.0)

            # --- input DMAs, ordered along the critical path
            wqk = cst.tile([128, NCH, NCH, 2, 128], F32R, tag="wqk")
            wqk_src = [
                d.ap().rearrange("(c p) (t e) -> p c t e", p=128, t=2)
                for d in (wqk0, wqk1)
            ]
            xr = cst.tile([128, NCH, S], F32R, tag="xr")
            xsrc = xin.ap().rearrange("(c p) s -> p c s", p=128)
            bt = cst.tile([128, 3, NCH], F32, tag="bt")
            msk2 = cst.tile([128, 256], BF16, tag="msk2")
            bprow = cst.tile([1, 256], BF16, tag="bprow")
            ones = cst.tile([1, 512], BF16, tag="ones")
            nc.vector.memset(ones, 1.0)
            wv = cst.tile([128, NCH, 256], F32R, tag="wv")
            wp = cst.tile([128, NCH, 256], BF16, tag="wp")

            nc.sync.dma_start(out=wqk[:, :, 0], in_=wqk_src[0])
            nc.sync.dma_start(out=xr[:, :, 0:256], in_=xsrc[:, :, 0:256])
            nc.sync.dma_start(
                out=bt, in_=biasd.ap().rearrange("(b m p) -> p b m", p=128, b=3)
            )
            nc.sync.dma_start(out=msk2, in_=msk2d.ap())
            nc.sync.dma_start(out=bprow, in_=bprowd.ap())
            nc.sync.dma_start(out=xr[:, :, 256:512], in_=xsrc[:, :, 256:512])
            nc.sync.dma_start(out=xr[:, :, 512:1024], in_=xsrc[:, :, 512:1024])
            nc.sync.dma_start(out=wqk[:, :, 1], in_=wqk_src[1])
            nc.sync.dma_start(out=wv, in_=wvt.ap().rearrange("(c p) e -> p c e", p=128))
            nc.sync.dma_start(out=wp, in_=wpt.ap().rearrange("(c p) e -> p c e", p=128))

            ident = msk2[:, 0:128]
            mlow = msk2[:, 128:256]

            q_sb = cst.tile([128, NCH, S], F32R, tag="q_sb")
            k_sb = cst.tile([128, NCH, S], F32R, tag="k_sb")
            att = cst.tile([128, NCH, S], BF16, tag="att")

            def qk_proj(t, m, c0, c1, eng):
                # t: 0 = q, 1 = k; columns [c0, c1); eng: 'dve' | 'act'
                # psum from the pa ring (same 2KB/partition slot size), which
                # is otherwise idle while projections run
                dst = (q_sb, k_sb)[t]
                pp = pap.tile([128, 512], F32, tag="pa")
                w = c1 - c0
                for c in range(2):
                    nc.tensor.matmul(
                        pp[:, 0:w],
                        wqk[:, c, m, t, :],
                        xr[:, c, c0:c1],
                        start=(c == 0), stop=(c == 1),
                    )
                if eng == 'act':
                    nc.scalar.add(dst[:, m, c0:c1], pp[:, 0:w], bt[:, t, m:m + 1])
                else:
                    nc.vector.tensor_scalar_add(
                        dst[:, m, c0:c1], pp[:, 0:w], bt[:, t, m:m + 1]
                    )

            pts_tiles = {}

            def get_pts(h):
                if h not in pts_tiles:
                    pts = ptp.tile([128, 8, S], BF16, tag="pts", name=f"pts{h}")
                    pts_tiles[h] = pts
                return pts_tiles[h]

            def sc_win(h, i, ps, off, j):
                # score matmuls for sq window j of sk chunk i (+ PE mask on
                # the diagonal block window)
                m, r = h // 4, h % 4
                rows = slice(32 * r, 32 * r + 32)
                we = 512 * (j + 1)
                if we <= 128 * i:
                    return
                ws = max(512 * j, 128 * i)
                ws_mm = max(min(ws, we - 256), 512 * j)
                diag = ws == 128 * i  # window containing the diagonal block
                nc.tensor.matmul(
                    ps[:, ws_mm - off:we - off],
                    k_sb[rows, m, 128 * i:128 * (i + 1)],
                    q_sb[rows, m, ws_mm:we],
                    start=True, stop=not diag,
                    tile_position=(32 * r, 0),
                )
                if diag:
                    # psum[sk, sq] += -1e30 for sq < sk within the block
                    nc.tensor.matmul(
                        ps[:, 128 * i - off:128 * (i + 1) - off],
                        ident, mlow,
                        start=False, stop=True,
                    )

            def sc_exp(h, i, ps, off, e0, e1):
                nc.scalar.activation(
                    get_pts(h)[:, i, e0:e1], ps[:, e0 - off:e1 - off], EXP
                )

            def sc(h, i, use_big=None):
                # full steady-state chunk: both windows + one exp
                get_pts(h)
                if use_big or i < 4:
                    ps = big.tile([128, S], F32, tag="bg", name=f"ps{h}_{i}")
                    off = 0
                else:
                    ps = sml.tile([128, 512], F32, tag="sm", name=f"ps{h}_{i}")
                    off = 512
                for j in range(2):
                    sc_win(h, i, ps, off, j)
                sc_exp(h, i, ps, off, 128 * i, S)

            def att_mms(h, pa, ii, q0, q1, first, last, base=0):
                # accumulate sq columns [q0, q1) into pa[:, q0-base:q1-base]
                pts = pts_tiles[h]
                for idx, i in enumerate(ii):
                    ws = max(q0, 128 * i)
                    nc.tensor.matmul(
                        pa[:, ws - base:q1 - base],
                        vaug[:, i, h, :],
                        pts[:, i, ws:q1],
                        start=(first and idx == 0),
                        stop=(last and idx == len(ii) - 1),
                    )

            def att_recip_bcast(pa, p0, p1):
                w = p1 - p0
                rf = rbp.tile([1, 512], F32, tag="rf")
                nc.vector.reciprocal(rf[:, 0:w], pa[32:33, p0:p1])
                rb = rbp.tile([32, 512], F32, tag="rb")
                nc.gpsimd.partition_broadcast(rb[:, 0:w], rf[:, 0:w])
                return rb

            def att_mul(h, pa, rb, q0, q1, p0=0):
                # att columns [q0, q1) normalized from pa[0:32, p0:p0+w]
                m, r = h // 4, h % 4
                w = q1 - q0
                nc.vector.tensor_mul(
                    att[32 * r:32 * r + 32, m, q0:q1],
                    pa[0:32, p0:p0 + w], rb[:, 0:w],
                )

            out_ap = outd.ap().rearrange("(m p) s -> p m s", p=128)

            def po_piece(q0, q1, eng):
                # output projection for columns [q0, q1); the bias is folded
                # in as a bias-row x ones accumulate matmul, so the psum ->
                # SBUF move is a single plain copy per piece
                w = q1 - q0
                po = big.tile([128, 2, 512], F32, tag="bg", name=f"po{q0}")
                for m in range(2):
                    for c in range(2):
                        nc.tensor.matmul(
                            po[:, m, 0:w],
                            wp[:, c, m * 128:(m + 1) * 128],
                            att[:, c, q0:q1],
                            start=(c == 0), stop=False,
                        )
                    nc.tensor.matmul(
                        po[:, m, 0:w],
                        bprow[:, m * 128:(m + 1) * 128],
                        ones[:, 0:w],
                        start=False, stop=True,
                    )
                ot = osb.tile([128, 2, 512], F32, tag="ot", name=f"ot{q0}")
                if eng == 'act':
                    nc.scalar.copy(ot[:, :, 0:w], po[:, :, 0:w])
                else:
                    nc.vector.tensor_copy(ot[:, :, 0:w], po[:, :, 0:w])
                nc.sync.dma_start(out=out_ap[:, :, q0:q1], in_=ot[:, :, 0:w])

            # ---------------- head 0 priming ----------------
            def sc_win0(i, ps, off, w0, w1):
                # head-0 score mms + exp for sq window [w0, w1) of chunk i
                ws = max(w0, 128 * i)
                if w1 <= ws:
                    return
                ws_mm = max(min(ws, w1 - 256), w0)
                diag = ws == 128 * i
                nc.tensor.matmul(
                    ps[:, ws_mm - off:w1 - off],
                    k_sb[0:32, 0, 128 * i:128 * (i + 1)],
                    q_sb[0:32, 0, ws_mm:w1],
                    start=True, stop=not diag,
                    tile_position=(0, 0),
                )
                if diag:
                    nc.tensor.matmul(
                        ps[:, 128 * i - off:128 * (i + 1) - off],
                        ident, mlow,
                        start=False, stop=True,
                    )
                nc.scalar.activation(
                    get_pts(0)[:, i, ws:w1], ps[:, ws - off:w1 - off], EXP
                )

            qk_proj(0, 0, 0, 256, 'dve')
            qk_proj(1, 0, 0, 256, 'act')
            b0 = big.tile([128, S], F32, tag="bg", name="ps0_0")
            b1 = big.tile([128, S], F32, tag="bg", name="ps0_1")
            sc_win0(0, b0, 0, 0, 256)
            sc_win0(1, b1, 0, 0, 256)
            qk_proj(0, 0, 256, 512, 'dve')
            qk_proj(1, 0, 256, 512, 'act')
            sc_win0(0, b0, 0, 256, 512)
            sc_win0(1, b1, 0, 256, 512)
            s2 = sml.tile([128, 512], F32, tag="sm", name="ps0_2a")
            sc_win0(2, s2, 256, 256, 512)
            s3 = sml.tile([128, 512], F32, tag="sm", name="ps0_3a")
            sc_win0(3, s3, 256, 256, 512)
            # j1 windows (x second half)
            qk_proj(0, 0, 512, 1024, 'dve')
            qk_proj(1, 0, 512, 1024, 'act')
            sc_win0(0, b0, 0, 512, 1024)
            sc_win0(1, b1, 0, 512, 1024)
            s2b = sml.tile([128, 512], F32, tag="sm", name="ps0_2b")
            sc_win0(2, s2b, 512, 512, 1024)
            s3b = sml.tile([128, 512], F32, tag="sm", name="ps0_3b")
            sc_win0(3, s3b, 512, 512, 1024)
            for i in (4, 5, 6):
                sc(0, i)
            sc(1, 0)
            sc(0, 7)

            # v projection (after wv lands) + head-0 attnv j0
            def v_proj(i):
                pv = pap.tile([128, 512], F32, tag="pa", name=f"pv{i}")
                for c in range(2):
                    nc.tensor.matmul(
                        pv[:, 0:256],
                        xr[:, c, i * 128:(i + 1) * 128],
                        wv[:, c, :],
                        start=(c == 0), stop=(c == 1),
                    )
                nc.vector.tensor_copy(
                    vaug[:, i, :, 0:D],
                    pv[:, 0:256].rearrange("p (h d) -> p h d", h=H),
                )

            # m1 q/k projections: wqk1 lands mid-head-0; doing these here
            # keeps their psum slots and bias-adds off the steady-state path
            qk_proj(0, 1, 0, 512, 'dve')
            qk_proj(1, 1, 0, 512, 'dve')
            qk_proj(0, 1, 512, 1024, 'dve')
            qk_proj(1, 1, 512, 1024, 'dve')

            for i in range(8):
                v_proj(i)

            # One [33,512] accumulator per head: j0 round in head h, then the
            # j1 round REUSES the same tile in h+1 (after the j0 multiply)
            pa_att = {}  # h -> accumulator tile
            pa_j0 = {}   # h -> rb for the j0 half

            def attn_j0(h):
                pa = pap.tile([33, 512], F32, tag="pa", name=f"pa{h}")
                pa_att[h] = pa
                att_mms(h, pa, [0, 1, 2, 3], 0, 512, True, True)
                pa_j0[h] = att_recip_bcast(pa, 0, 512)

            attn_j0(0)
            att_mul(0, pa_att[0], pa_j0.pop(0), 0, 512)

            # ---------------- steady heads ----------------
            def emit_steady(h):
                prev = h - 1
                for i in (1, 2, 3, 4, 5):
                    sc(h, i)
                # attnv j0 of h: mms ready after e3; PE reaches them here
                attn_j0(h)
                sc(h + 1, 0)
                sc(h, 6)
                sc(h, 7)
                # j0 multiply of h (dep: bcast just emitted)
                att_mul(h, pa_att[h], pa_j0.pop(h), 0, 512)
                # attnv j1 of prev, reusing its accumulator (j0 mul done)
                pa_prev = pa_att.pop(prev)
                att_mms(prev, pa_prev, [0, 1, 2, 3], 512, 1024, True, False,
                        base=512)
                att_mms(prev, pa_prev, [4, 5, 6, 7], 512, 1024, False, True,
                        base=512)
                rbj1 = att_recip_bcast(pa_prev, 0, 512)
                att_mul(prev, pa_prev, rbj1, 512, 1024, p0=0)
                pts_tiles.pop(prev)

            for h in range(1, 7):
                emit_steady(h)

            # ---------------- head 7 + tail ----------------
            p7 = get_pts(7)
            pa6 = pa_att.pop(6)
            sc(7, 1)
            sc(7, 2)
            # j1 of head 6, interleaved with head-7 scores
            att_mms(6, pa6, [0, 1], 512, 1024, True, False, base=512)
            sc(7, 3)
            att_mms(6, pa6, [2, 3], 512, 1024, False, False, base=512)
            sc(7, 4)
            # attnv j0 of head 7, group A: cols [0:256) needs chunks 0,1
            pa7 = pap.tile([33, 512], F32, tag="pa", name="pa7")
            nc.tensor.matmul(pa7[:, 0:256], vaug[:, 0, 7, :], p7[:, 0, 0:256],
                             start=True, stop=False)
            nc.tensor.matmul(pa7[:, 128:256], vaug[:, 1, 7, :], p7[:, 1, 128:256],
                             start=False, stop=True)
            rb_a = att_recip_bcast(pa7, 0, 256)
            sc(7, 5)
            att_mms(6, pa6, [4, 5, 6, 7], 512, 1024, False, True, base=512)
            rbj1_6 = att_recip_bcast(pa6, 0, 512)
            # group B: cols [256:512) needs chunks 0-3
            nc.tensor.matmul(pa7[:, 256:512], vaug[:, 0, 7, :], p7[:, 0, 256:512],
                             start=True, stop=False)
            nc.tensor.matmul(pa7[:, 256:512], vaug[:, 1, 7, :], p7[:, 1, 256:512],
                             start=False, stop=False)
            nc.tensor.matmul(pa7[:, 256:512], vaug[:, 2, 7, :], p7[:, 2, 256:512],
                             start=False, stop=False)
            nc.tensor.matmul(pa7[:, 384:512], vaug[:, 3, 7, :], p7[:, 3, 384:512],
                             start=False, stop=True)
            rb_b = att_recip_bcast(pa7, 256, 512)
            # chunks 6/7 ride the big ring in head 7 (no e0' slot in the
            # exp order, so the small-ring WAR deadlines would be tight)
            sc(7, 6, use_big=True)
            sc(7, 7, use_big=True)
            pts_tiles.pop(6)
            # tail accumulators first: [512:768) and [768:1024) pieces on the
            # small ring (WAR on chunks 4/5's exps - early)
            pa_c = sml.tile([33, 512], F32, tag="sm", name="pa_c")
            att_mms(7, pa_c, [0, 1, 2, 3, 4, 5], 512, 768, True, True, base=512)
            pa_de = sml.tile([33, 512], F32, tag="sm", name="pa_de")
            att_mms(7, pa_de, [0, 1, 2, 3, 4, 5], 768, 1024, True, False,
                    base=768)
            # muls + outproj pieces, pipelined against the exp tail
            att_mul(7, pa7, rb_a, 0, 256)
            po_piece(0, 256, 'act')
            att_mms(7, pa_de, [6], 768, 1024, False, False, base=768)
            rb_c = att_recip_bcast(pa_c, 0, 256)
            att_mul(6, pa6, rbj1_6, 512, 1024, p0=0)
            att_mul(7, pa7, rb_b, 256, 512, p0=256)
            po_piece(256, 512, 'act')
            att_mms(7, pa_de, [7], 768, 1024, False, True, base=768)
            rb_de = att_recip_bcast(pa_de, 0, 256)
            att_mul(7, pa_c, rb_c, 512, 768, p0=0)
            po_piece(512, 768, 'act')
            att_mul(7, pa_de, rb_de, 768, 1024, p0=0)
            po_piece(768, 1024, 'act')

    nc.compile()
    return nc


def get_program():
    if "nc" not in _CACHE:
        _CACHE["nc"] = _build_program()
    return _CACHE["nc"]


def kernel(x, wq, bq, wkv, bkv, wproj, bproj):
    import ml_dtypes
    from concourse.bass_utils import run_bass_kernel_spmd

    nc = get_program()

    x = np.asarray(x, dtype=np.float32)
    n = x.shape[0]
    assert n == N_CORES and x.shape[1:] == (C, 32, 32)

    scale = 1.0 / np.sqrt(np.float32(D))
    wq_s = np.asarray(wq, np.float32) * scale
    bq_s = np.asarray(bq, np.float32) * scale
    wk = np.asarray(wkv[:E], np.float32)
    bk = np.asarray(bkv[:E], np.float32)
    wv = np.asarray(wkv[E:], np.float32)
    bv = np.asarray(bkv[E:], np.float32)
    wproj = np.asarray(wproj, np.float32)
    bproj_eff = (np.asarray(bproj, np.float32)
                 + wproj.astype(np.float64) @ bv.astype(np.float64)).astype(np.float32)

    # msk2: [identity | -1e30 * strict_lower(r > sq)]
    ident = np.eye(128, dtype=np.float32)
    mlow = np.where(np.arange(128)[:, None] > np.arange(128)[None, :],
                    np.float32(-1e30), np.float32(0.0))
    msk2 = np.concatenate([ident, mlow], axis=1).astype(ml_dtypes.bfloat16)

    shared = {
        "wqk0": np.ascontiguousarray(
            np.concatenate([wq_s.T[:, 0:128], wk.T[:, 0:128]], axis=1)),
        "wqk1": np.ascontiguousarray(
            np.concatenate([wq_s.T[:, 128:256], wk.T[:, 128:256]], axis=1)),
        "wvt": np.ascontiguousarray(wv.T),
        "wpt": np.ascontiguousarray(wproj.T.astype(ml_dtypes.bfloat16)),
        "biasd": np.ascontiguousarray(
            np.concatenate([bq_s, bk, bproj_eff])),
        "bprow": np.ascontiguousarray(
            bproj_eff.reshape(1, 256).astype(ml_dtypes.bfloat16)),
        "msk2": np.ascontiguousarray(msk2),
    }
    in_maps = [
        {"xin": np.ascontiguousarray(x[i].reshape(C, S)), **shared}
        for i in range(N_CORES)
    ]
    res = run_bass_kernel_spmd(nc, in_maps, core_ids=list(range(N_CORES)))
    out = np.stack([res.results[i]["out"].reshape(O, 32, 32) for i in range(N_CORES)])
    return out.astype(np.float32)


# revision 53
# speedup vs baseline: 1.1390x; 1.0280x over previous
"""Causal multi-head attention (8 heads, 1x1-conv projections) on 8 TRN2 cores.

Sharding: data-parallel over batch N=8 -> one batch element per NeuronCore.
Per-core kernel (S=1024 pixels, C=E=256 channels, H=8 heads, d=32):
  q = WqT.T @ x, k = WkT.T @ x              (e, s) layout, fp32r matmuls
  vT = x.T @ WvT                            (s, e) layout
  per head: P^T[sk, sq] = exp(mask(k_h^T q_h))   scores computed TRANSPOSED so
                                            softmax denominator comes from an
                                            appended ones-column in v (M=33)
  out_h = (vAug_h^T @ P^T) -> rows 0..31 numerator^T, row 32 = denominator
  att = num / denom (bf16), out = WprojT.T @ att + bproj_eff

Design (v2, ACT-stream-paced):
- The exp stream on the Activation engine is the pacing resource (~44us of
  exp work).  Everything else is scheduled around keeping it gap-free.
- Causal masking is folded into the score matmuls: an extra bf16 accumulate
  matmul (identity stationary x strictly-lower -BIG moving) writes -1e30
  into the masked diagonal-block region of PSUM before the exp, so exp
  emits exact zeros and the DVE never touches masks.
- Per head h: attnv j0-half accumulates during head h (after exp chunk 3),
  the j1-half during head h+1.  Softmax normalization: DVE reciprocal of
  the denominator row, gpsimd partition_broadcast, deferred DVE multiply.
- exp emission order per head: e1..e5, e0(next head), e6, e7 -- psum-ring
  write-after-read hazards then never block a score matmul near its exp.
- PSUM: 2x[128,1024] ring (chunks 0-3 + outproj), 2x[128,512] ring
  (chunks 4-7 + q/k/v projections), 2x[33,512] ring (attnv accumulators).
- Output projection + DMA run in 4 column pieces ([0:256], [256:512],
  [512:768], [768:1024]) pipelined against the last head's exp tail; att
  and Wproj are bf16 so narrow matmuls run at full PE rate.
- Startup: x[:, 0:512] lands early (one DMA queue, transfers serialize),
  PE warmup matmuls start at ~250ns so the PE clock is ramped before the
  first projection; head 0 primes the stream with j0-window exps.
"""

import numpy as np

N_CORES = 8
C = 256      # input channels
E = 256      # embed channels (q/k)
O = 256      # v/out channels
S = 1024     # spatial positions (32*32)
H = 8        # heads
D = 32       # head dim
NCH = 2      # 256 = 2 * 128 partition chunks
N_WARM = 33  # PE clock-warm matmuls

_CACHE = {}


def _build_program():
    import concourse.mybir as mybir
    from concourse import bacc
    from concourse import library_config
    from concourse.tile import TileContext

    F32 = mybir.dt.float32
    F32R = mybir.dt.float32r
    BF16 = mybir.dt.bfloat16
    EXP = mybir.ActivationFunctionType.Exp

    nc = bacc.Bacc("TRN2", target_bir_lowering=False, debug=False)

    xin = nc.dram_tensor("xin", [C, S], F32R, kind="ExternalInput")
    wqk0 = nc.dram_tensor("wqk0", [C, 2 * 128], F32R, kind="ExternalInput")
    wqk1 = nc.dram_tensor("wqk1", [C, 2 * 128], F32R, kind="ExternalInput")
    wvt = nc.dram_tensor("wvt", [C, O], F32R, kind="ExternalInput")
    wpt = nc.dram_tensor("wpt", [O, O], BF16, kind="ExternalInput")
    biasd = nc.dram_tensor("biasd", [3 * 256], F32, kind="ExternalInput")
    bprowd = nc.dram_tensor("bprow", [1, 256], BF16, kind="ExternalInput")
    # msk2[:, 0:128] = identity, msk2[:, 128:256] = -1e30 * strict_lower
    msk2d = nc.dram_tensor("msk2", [128, 256], BF16, kind="ExternalInput")
    outd = nc.dram_tensor("out", [O, S], F32, kind="ExternalOutput")

    with TileContext(nc) as tc:
        with (
            tc.tile_pool(name="cst", bufs=1) as cst,
            tc.tile_pool(name="ptp", bufs=3) as ptp,
            tc.tile_pool(name="rbp", bufs=4) as rbp,
            tc.tile_pool(name="osb", bufs=4) as osb,
            tc.tile_pool(name="big", bufs=2, space="PSUM") as big,
            tc.tile_pool(name="sml", bufs=2, space="PSUM") as sml,
            tc.tile_pool(name="pap", bufs=2, space="PSUM") as pap,
        ):
            # --- fast start: Pool memsets, ACT table preload, PE clock warm
            wup = cst.tile([128, 64], F32, tag="wup")
            nc.gpsimd.memset(wup, 0.0)
            dmz = cst.tile([128, 1], F32, tag="dmz")
            nc.gpsimd.memset(dmz, 0.0)
            nc.gpsimd.load_library(library_config.attn)
            dme = cst.tile([128, 1], F32, tag="dme")
            nc.scalar.activation(dme, dmz, EXP)
            pwu = big.tile([64, 512], F32, tag="bg")
            for _ in range(N_WARM):
                nc.tensor.matmul(pwu[:, 0:64], wup.bitcast(F32R),
                                 wup[:, 0:64].bitcast(F32R),
                                 start=True, stop=True)

            vaug = cst.tile([128, 8, H, D + 1], BF16, tag="vaug")
            nc.vector.memset(vaug[:, :, :, D], 1.0)

            # --- input DMAs, ordered along the critical path
            wqk = cst.tile([128, NCH, NCH, 2, 128], F32R, tag="wqk")
            wqk_src = [
                d.ap().rearrange("(c p) (t e) -> p c t e", p=128, t=2)
                for d in (wqk0, wqk1)
            ]
            xr = cst.tile([128, NCH, S], F32R, tag="xr")
            xsrc = xin.ap().rearrange("(c p) s -> p c s", p=128)
            bt = cst.tile([128, 3, NCH], F32, tag="bt")
            msk2 = cst.tile([128, 256], BF16, tag="msk2")
            bprow = cst.tile([1, 256], BF16, tag="bprow")
            ones = cst.tile([1, 512], BF16, tag="ones")
            nc.vector.memset(ones, 1.0)
            wv = cst.tile([128, NCH, 256], F32R, tag="wv")
            wp = cst.tile([128, NCH, 256], BF16, tag="wp")

            nc.sync.dma_start(out=wqk[:, :, 0], in_=wqk_src[0])
            nc.sync.dma_start(out=xr[:, :, 0:256], in_=xsrc[:, :, 0:256])
            nc.sync.dma_start(
                out=bt, in_=biasd.ap().rearrange("(b m p) -> p b m", p=128, b=3)
            )
            nc.sync.dma_start(out=msk2, in_=msk2d.ap())
            nc.sync.dma_start(out=bprow, in_=bprowd.ap())
            nc.sync.dma_start(out=xr[:, :, 256:512], in_=xsrc[:, :, 256:512])
            nc.sync.dma_start(out=xr[:, :, 512:1024], in_=xsrc[:, :, 512:1024])
            nc.sync.dma_start(out=wqk[:, :, 1], in_=wqk_src[1])
            nc.sync.dma_start(out=wv, in_=wvt.ap().rearrange("(c p) e -> p c e", p=128))
            nc.sync.dma_start(out=wp, in_=wpt.ap().rearrange("(c p) e -> p c e", p=128))

            ident = msk2[:, 0:128]
            mlow = msk2[:, 128:256]

            q_sb = cst.tile([128, NCH, S], F32R, tag="q_sb")
            k_sb = cst.tile([128, NCH, S], F32R, tag="k_sb")
            att = cst.tile([128, NCH, S], BF16, tag="att")

            def qk_proj(t, m, c0, c1, eng):
                # t: 0 = q, 1 = k; columns [c0, c1); eng: 'dve' | 'act'
                # psum from the pa ring (same 2KB/partition slot size), which
                # is otherwise idle while projections run
                dst = (q_sb, k_sb)[t]
                pp = pap.tile([128, 512], F32, tag="pa")
                w = c1 - c0
                for c in range(2):
                    nc.tensor.matmul(
                        pp[:, 0:w],
                        wqk[:, c, m, t, :],
                        xr[:, c, c0:c1],
                        start=(c == 0), stop=(c == 1),
                    )
                if eng == 'act':
                    nc.scalar.add(dst[:, m, c0:c1], pp[:, 0:w], bt[:, t, m:m + 1])
                else:
                    nc.vector.tensor_scalar_add(
                        dst[:, m, c0:c1], pp[:, 0:w], bt[:, t, m:m + 1]
                    )

            # pts is PACKED: chunk i's live columns [128i, S) stored at
            # [POFF[i], POFF[i+1]), so merged exps write contiguous ranges
            POFF = [0, 1024, 1920, 2688, 3328, 3840, 4224, 4480, 4608]
            pts_tiles = {}

            def get_pts(h):
                if h not in pts_tiles:
                    pts = ptp.tile([128, POFF[8]], BF16, tag="pts",
                                   name=f"pts{h}")
                    pts_tiles[h] = pts
                return pts_tiles[h]

            def pcol(i, c):
                # pts column for chunk i, sq position c
                return POFF[i] + c - 128 * i

            def sc_win(h, i, ps, off, j):
                # score matmuls for sq window j of sk chunk i (+ PE mask on
                # the diagonal block window)
                m, r = h // 4, h % 4
                rows = slice(32 * r, 32 * r + 32)
                we = 512 * (j + 1)
                if we <= 128 * i:
                    return
                ws = max(512 * j, 128 * i)
                ws_mm = max(min(ws, we - 256), 512 * j)
                diag = ws == 128 * i  # window containing the diagonal block
                nc.tensor.matmul(
                    ps[:, ws_mm - off:we - off],
                    k_sb[rows, m, 128 * i:128 * (i + 1)],
                    q_sb[rows, m, ws_mm:we],
                    start=True, stop=not diag,
                    tile_position=(32 * r, 0),
                )
                if diag:
                    # psum[sk, sq] += -1e30 for sq < sk within the block
                    nc.tensor.matmul(
                        ps[:, 128 * i - off:128 * (i + 1) - off],
                        ident, mlow,
                        start=False, stop=True,
                    )

            def sc_exp(h, i, ps, off, e0, e1):
                nc.scalar.activation(
                    get_pts(h)[:, pcol(i, e0):pcol(i, e1)],
                    ps[:, e0 - off:e1 - off], EXP
                )

            def sc(h, i):
                # steady-state chunk 0-3: both windows + one exp (big ring)
                get_pts(h)
                ps = big.tile([128, S], F32, tag="bg", name=f"ps{h}_{i}")
                for j in range(2):
                    sc_win(h, i, ps, 0, j)
                sc_exp(h, i, ps, 0, 128 * i, S)

            def sc45(h):
                # chunks 4+5 share a big tile (c4 at [0:512], c5 at
                # [512:896]) and ONE exp over the packed pts range
                m, r = h // 4, h % 4
                rows = slice(32 * r, 32 * r + 32)
                ps = big.tile([128, S], F32, tag="bg", name=f"ps{h}_45")
                nc.tensor.matmul(ps[:, 0:512], k_sb[rows, m, 512:640],
                                 q_sb[rows, m, 512:1024], start=True,
                                 stop=False, tile_position=(32 * r, 0))
                nc.tensor.matmul(ps[:, 0:128], ident, mlow,
                                 start=False, stop=True)
                nc.tensor.matmul(ps[:, 512:896], k_sb[rows, m, 640:768],
                                 q_sb[rows, m, 640:1024], start=True,
                                 stop=False, tile_position=(32 * r, 0))
                nc.tensor.matmul(ps[:, 512:640], ident, mlow,
                                 start=False, stop=True)
                nc.scalar.activation(
                    get_pts(h)[:, POFF[4]:POFF[6]], ps[:, 0:896], EXP)

            def sc67(h):
                # chunks 6+7 share a small tile (c6 at [0:256], c7 at
                # [256:384], unwidened) and ONE exp
                m, r = h // 4, h % 4
                rows = slice(32 * r, 32 * r + 32)
                ps = sml.tile([128, 512], F32, tag="sm", name=f"ps{h}_67")
                nc.tensor.matmul(ps[:, 0:256], k_sb[rows, m, 768:896],
                                 q_sb[rows, m, 768:1024], start=True,
                                 stop=False, tile_position=(32 * r, 0))
                nc.tensor.matmul(ps[:, 0:128], ident, mlow,
                                 start=False, stop=True)
                nc.tensor.matmul(ps[:, 256:384], k_sb[rows, m, 896:1024],
                                 q_sb[rows, m, 896:1024], start=True,
                                 stop=False, tile_position=(32 * r, 0))
                nc.tensor.matmul(ps[:, 256:384], ident, mlow,
                                 start=False, stop=True)
                nc.scalar.activation(
                    get_pts(h)[:, POFF[6]:POFF[8]], ps[:, 0:384], EXP)

            def att_mms(h, pa, ii, q0, q1, first, last, base=0):
                # accumulate sq columns [q0, q1) into pa[:, q0-base:q1-base]
                pts = pts_tiles[h]
                for idx, i in enumerate(ii):
                    ws = max(q0, 128 * i)
                    nc.tensor.matmul(
                        pa[:, ws - base:q1 - base],
                        vaug[:, i, h, :],
                        pts[:, pcol(i, ws):pcol(i, q1)],
                        start=(first and idx == 0),
                        stop=(last and idx == len(ii) - 1),
                    )

            def att_recip_bcast(pa, p0, p1):
                w = p1 - p0
                rf = rbp.tile([1, 512], F32, tag="rf")
                nc.vector.reciprocal(rf[:, 0:w], pa[32:33, p0:p1])
                rb = rbp.tile([32, 512], F32, tag="rb")
                nc.gpsimd.partition_broadcast(rb[:, 0:w], rf[:, 0:w])
                return rb

            def att_mul(h, pa, rb, q0, q1, p0=0):
                # att columns [q0, q1) normalized from pa[0:32, p0:p0+w]
                m, r = h // 4, h % 4
                w = q1 - q0
                nc.vector.tensor_mul(
                    att[32 * r:32 * r + 32, m, q0:q1],
                    pa[0:32, p0:p0 + w], rb[:, 0:w],
                )

            out_ap = outd.ap().rearrange("(m p) s -> p m s", p=128)

            def po_piece(q0, q1, eng):
                # output projection for columns [q0, q1); the bias is folded
                # in as a bias-row x ones accumulate matmul, so the psum ->
                # SBUF move is a single plain copy per piece
                w = q1 - q0
                po = big.tile([128, 2, 512], F32, tag="bg", name=f"po{q0}")
                for m in range(2):
                    for c in range(2):
                        nc.tensor.matmul(
                            po[:, m, 0:w],
                            wp[:, c, m * 128:(m + 1) * 128],
                            att[:, c, q0:q1],
                            start=(c == 0), stop=False,
                        )
                    nc.tensor.matmul(
                        po[:, m, 0:w],
                        bprow[:, m * 128:(m + 1) * 128],
                        ones[:, 0:w],
                        start=False, stop=True,
                    )
                ot = osb.tile([128, 2, 512], F32, tag="ot", name=f"ot{q0}")
                if eng == 'act':
                    nc.scalar.copy(ot[:, :, 0:w], po[:, :, 0:w])
                else:
                    nc.vector.tensor_copy(ot[:, :, 0:w], po[:, :, 0:w])
                nc.sync.dma_start(out=out_ap[:, :, q0:q1], in_=ot[:, :, 0:w])

            # ---------------- head 0 priming ----------------
            def sc_win0(i, ps, off, w0, w1):
                # head-0 score mms + exp for sq window [w0, w1) of chunk i
                ws = max(w0, 128 * i)
                if w1 <= ws:
                    return
                ws_mm = max(min(ws, w1 - 256), w0)
                diag = ws == 128 * i
                nc.tensor.matmul(
                    ps[:, ws_mm - off:w1 - off],
                    k_sb[0:32, 0, 128 * i:128 * (i + 1)],
                    q_sb[0:32, 0, ws_mm:w1],
                    start=True, stop=not diag,
                    tile_position=(0, 0),
                )
                if diag:
                    nc.tensor.matmul(
                        ps[:, 128 * i - off:128 * (i + 1) - off],
                        ident, mlow,
                        start=False, stop=True,
                    )
                nc.scalar.activation(
                    get_pts(0)[:, pcol(i, ws):pcol(i, w1)],
                    ps[:, ws - off:w1 - off], EXP
                )

            qk_proj(0, 0, 0, 256, 'dve')
            qk_proj(1, 0, 0, 256, 'act')
            b0 = big.tile([128, S], F32, tag="bg", name="ps0_0")
            b1 = big.tile([128, S], F32, tag="bg", name="ps0_1")
            sc_win0(0, b0, 0, 0, 256)
            sc_win0(1, b1, 0, 0, 256)
            qk_proj(0, 0, 256, 512, 'dve')
            qk_proj(1, 0, 256, 512, 'act')
            sc_win0(0, b0, 0, 256, 512)
            sc_win0(1, b1, 0, 256, 512)
            s2 = sml.tile([128, 512], F32, tag="sm", name="ps0_2a")
            sc_win0(2, s2, 256, 256, 512)
            s3 = sml.tile([128, 512], F32, tag="sm", name="ps0_3a")
            sc_win0(3, s3, 256, 256, 512)
            # j1 windows (x second half)
            qk_proj(0, 0, 512, 1024, 'dve')
            qk_proj(1, 0, 512, 1024, 'act')
            sc_win0(0, b0, 0, 512, 1024)
            sc_win0(1, b1, 0, 512, 1024)
            s2b = sml.tile([128, 512], F32, tag="sm", name="ps0_2b")
            sc_win0(2, s2b, 512, 512, 1024)
            s3b = sml.tile([128, 512], F32, tag="sm", name="ps0_3b")
            sc_win0(3, s3b, 512, 512, 1024)
            sc45(0)
            sc(1, 0)
            sc67(0)

            # v projection (after wv lands) + head-0 attnv j0
            def v_proj(i):
                pv = pap.tile([128, 512], F32, tag="pa", name=f"pv{i}")
                for c in range(2):
                    nc.tensor.matmul(
                        pv[:, 0:256],
                        xr[:, c, i * 128:(i + 1) * 128],
                        wv[:, c, :],
                        start=(c == 0), stop=(c == 1),
                    )
                nc.vector.tensor_copy(
                    vaug[:, i, :, 0:D],
                    pv[:, 0:256].rearrange("p (h d) -> p h d", h=H),
                )

            # m1 q/k projections: wqk1 lands mid-head-0; doing these here
            # keeps their psum slots and bias-adds off the steady-state path
            qk_proj(0, 1, 0, 512, 'dve')
            qk_proj(1, 1, 0, 512, 'dve')
            qk_proj(0, 1, 512, 1024, 'dve')
            qk_proj(1, 1, 512, 1024, 'dve')

            for i in range(8):
                v_proj(i)

            # One [33,512] accumulator per head: j0 round in head h, then the
            # j1 round REUSES the same tile in h+1 (after the j0 multiply)
            pa_att = {}  # h -> accumulator tile
            pa_j0 = {}   # h -> rb for the j0 half

            def attn_j0(h):
                pa = pap.tile([33, 512], F32, tag="pa", name=f"pa{h}")
                pa_att[h] = pa
                att_mms(h, pa, [0, 1, 2, 3], 0, 512, True, True)
                pa_j0[h] = att_recip_bcast(pa, 0, 512)

            attn_j0(0)
            att_mul(0, pa_att[0], pa_j0.pop(0), 0, 512)

            # ---------------- steady heads ----------------
            def emit_steady(h):
                prev = h - 1
                for i in (1, 2, 3):
                    sc(h, i)
                sc45(h)
                # attnv j0 of h: mms ready after e3; PE reaches them here
                attn_j0(h)
                sc(h + 1, 0)
                sc67(h)
                # j0 multiply of h (dep: bcast just emitted)
                att_mul(h, pa_att[h], pa_j0.pop(h), 0, 512)
                # attnv j1 of prev, reusing its accumulator (j0 mul done)
                pa_prev = pa_att.pop(prev)
                att_mms(prev, pa_prev, [0, 1, 2, 3], 512, 1024, True, False,
                        base=512)
                att_mms(prev, pa_prev, [4, 5, 6, 7], 512, 1024, False, True,
                        base=512)
                rbj1 = att_recip_bcast(pa_prev, 0, 512)
                att_mul(prev, pa_prev, rbj1, 512, 1024, p0=0)
                pts_tiles.pop(prev)

            for h in range(1, 7):
                emit_steady(h)

            # ---------------- head 7 + tail ----------------
            p7 = get_pts(7)
            pa6 = pa_att.pop(6)
            sc(7, 1)
            sc(7, 2)
            # j1 of head 6, interleaved with head-7 scores
            att_mms(6, pa6, [0, 1], 512, 1024, True, False, base=512)
            sc(7, 3)
            att_mms(6, pa6, [2, 3], 512, 1024, False, False, base=512)
            sc45(7)
            # attnv j0 of head 7, group A: cols [0:256) needs chunks 0,1
            pa7 = pap.tile([33, 512], F32, tag="pa", name="pa7")
            nc.tensor.matmul(pa7[:, 0:256], vaug[:, 0, 7, :], p7[:, 0:256],
                             start=True, stop=False)
            nc.tensor.matmul(pa7[:, 128:256], vaug[:, 1, 7, :],
                             p7[:, pcol(1, 128):pcol(1, 256)],
                             start=False, stop=True)
            rb_a = att_recip_bcast(pa7, 0, 256)
            att_mms(6, pa6, [4, 5, 6, 7], 512, 1024, False, True, base=512)
            rbj1_6 = att_recip_bcast(pa6, 0, 512)
            # group B: cols [256:512) needs chunks 0-3
            nc.tensor.matmul(pa7[:, 256:512], vaug[:, 0, 7, :], p7[:, 256:512],
                             start=True, stop=False)
            nc.tensor.matmul(pa7[:, 256:512], vaug[:, 1, 7, :],
                             p7[:, pcol(1, 256):pcol(1, 512)],
                             start=False, stop=False)
            nc.tensor.matmul(pa7[:, 256:512], vaug[:, 2, 7, :],
                             p7[:, pcol(2, 256):pcol(2, 512)],
                             start=False, stop=False)
            nc.tensor.matmul(pa7[:, 384:512], vaug[:, 3, 7, :],
                             p7[:, pcol(3, 384):pcol(3, 512)],
                             start=False, stop=True)
            rb_b = att_recip_bcast(pa7, 256, 512)
            sc67(7)
            pts_tiles.pop(6)
            # tail accumulators: [512:768) at cols [0:256) and [768:1024)
            # at cols [256:512) of one shared small-ring tile
            pa_cde = sml.tile([33, 512], F32, tag="sm", name="pa_cde")
            att_mms(7, pa_cde, [0, 1, 2, 3, 4, 5], 512, 768, True, True,
                    base=512)
            att_mms(7, pa_cde, [0, 1, 2, 3, 4, 5], 768, 1024, True, False,
                    base=512)
            # muls + outproj pieces, pipelined against the exp tail
            att_mul(7, pa7, rb_a, 0, 256)
            po_piece(0, 256, 'act')
            rb_c = att_recip_bcast(pa_cde, 0, 256)
            att_mul(6, pa6, rbj1_6, 512, 1024, p0=0)
            att_mul(7, pa7, rb_b, 256, 512, p0=256)
            po_piece(256, 512, 'act')
            att_mms(7, pa_cde, [6, 7], 768, 1024, False, True, base=512)
            rb_de = att_recip_bcast(pa_cde, 256, 512)
            att_mul(7, pa_cde, rb_c, 512, 768, p0=0)
            po_piece(512, 768, 'act')
            att_mul(7, pa_cde, rb_de, 768, 1024, p0=256)
            po_piece(768, 1024, 'act')

    nc.compile()
    return nc


def get_program():
    if "nc" not in _CACHE:
        _CACHE["nc"] = _build_program()
    return _CACHE["nc"]


def kernel(x, wq, bq, wkv, bkv, wproj, bproj):
    import ml_dtypes
    from concourse.bass_utils import run_bass_kernel_spmd

    nc = get_program()

    x = np.asarray(x, dtype=np.float32)
    n = x.shape[0]
    assert n == N_CORES and x.shape[1:] == (C, 32, 32)

    scale = 1.0 / np.sqrt(np.float32(D))
    wq_s = np.asarray(wq, np.float32) * scale
    bq_s = np.asarray(bq, np.float32) * scale
    wk = np.asarray(wkv[:E], np.float32)
    bk = np.asarray(bkv[:E], np.float32)
    wv = np.asarray(wkv[E:], np.float32)
    bv = np.asarray(bkv[E:], np.float32)
    wproj = np.asarray(wproj, np.float32)
    bproj_eff = (np.asarray(bproj, np.float32)
                 + wproj.astype(np.float64) @ bv.astype(np.float64)).astype(np.float32)

    # msk2: [identity | -1e30 * strict_lower(r > sq)]
    ident = np.eye(128, dtype=np.float32)
    mlow = np.where(np.arange(128)[:, None] > np.arange(128)[None, :],
                    np.float32(-1e30), np.float32(0.0))
    msk2 = np.concatenate([ident, mlow], axis=1).astype(ml_dtypes.bfloat16)

    shared = {
        "wqk0": np.ascontiguousarray(
            np.concatenate([wq_s.T[:, 0:128], wk.T[:, 0:128]], axis=1)),
        "wqk1": np.ascontiguousarray(
            np.concatenate([wq_s.T[:, 128:256], wk.T[:, 128:256]], axis=1)),
        "wvt": np.ascontiguousarray(wv.T),
        "wpt": np.ascontiguousarray(wproj.T.astype(ml_dtypes.bfloat16)),
        "biasd": np.ascontiguousarray(
            np.concatenate([bq_s, bk, bproj_eff])),
        "bprow": np.ascontiguousarray(
            bproj_eff.reshape(1, 256).astype(ml_dtypes.bfloat16)),
        "msk2": np.ascontiguousarray(msk2),
    }
    in_maps = [
        {"xin": np.ascontiguousarray(x[i].reshape(C, S)), **shared}
        for i in range(N_CORES)
    ]
    res = run_bass_kernel_spmd(nc, in_maps, core_ids=list(range(N_CORES)))
    out = np.stack([res.results[i]["out"].reshape(O, 32, 32) for i in range(N_CORES)])
    return out.astype(np.float32)


# revision 59
# speedup vs baseline: 1.1503x; 1.0100x over previous
"""Causal multi-head attention (8 heads, 1x1-conv projections) on 8 TRN2 cores.

Sharding: data-parallel over batch N=8 -> one batch element per NeuronCore.
Per-core kernel (S=1024 pixels, C=E=256 channels, H=8 heads, d=32):
  q = WqT.T @ x, k = WkT.T @ x              (e, s) layout, fp32r matmuls
  vT = x.T @ WvT                            (s, e) layout
  per head: P^T[sk, sq] = exp(mask(k_h^T q_h))   scores computed TRANSPOSED so
                                            softmax denominator comes from an
                                            appended ones-column in v (M=33)
  out_h = (vAug_h^T @ P^T) -> rows 0..31 numerator^T, row 32 = denominator
  att = num / denom (bf16), out = WprojT.T @ att + bproj_eff

Design (v2, ACT-stream-paced):
- The exp stream on the Activation engine is the pacing resource (~44us of
  exp work).  Everything else is scheduled around keeping it gap-free.
- Causal masking is folded into the score matmuls: an extra bf16 accumulate
  matmul (identity stationary x strictly-lower -BIG moving) writes -1e30
  into the masked diagonal-block region of PSUM before the exp, so exp
  emits exact zeros and the DVE never touches masks.
- Per head h: attnv j0-half accumulates during head h (after exp chunk 3),
  the j1-half during head h+1.  Softmax normalization: DVE reciprocal of
  the denominator row, gpsimd partition_broadcast, deferred DVE multiply.
- exp emission order per head: e1..e5, e0(next head), e6, e7 -- psum-ring
  write-after-read hazards then never block a score matmul near its exp.
- PSUM: 2x[128,1024] ring (chunks 0-3 + outproj), 2x[128,512] ring
  (chunks 4-7 + q/k/v projections), 2x[33,512] ring (attnv accumulators).
- Output projection + DMA run in 4 column pieces ([0:256], [256:512],
  [512:768], [768:1024]) pipelined against the last head's exp tail; att
  and Wproj are bf16 so narrow matmuls run at full PE rate.
- Startup: x[:, 0:512] lands early (one DMA queue, transfers serialize),
  PE warmup matmuls start at ~250ns so the PE clock is ramped before the
  first projection; head 0 primes the stream with j0-window exps.
"""

import numpy as np

N_CORES = 8
C = 256      # input channels
E = 256      # embed channels (q/k)
O = 256      # v/out channels
S = 1024     # spatial positions (32*32)
H = 8        # heads
D = 32       # head dim
NCH = 2      # 256 = 2 * 128 partition chunks
N_WARM = 33  # PE clock-warm matmuls

_CACHE = {}


def _build_program():
    import concourse.mybir as mybir
    from concourse import bacc
    from concourse import library_config
    from concourse.tile import TileContext

    F32 = mybir.dt.float32
    F32R = mybir.dt.float32r
    BF16 = mybir.dt.bfloat16
    EXP = mybir.ActivationFunctionType.Exp

    nc = bacc.Bacc("TRN2", target_bir_lowering=False, debug=False)

    xin = nc.dram_tensor("xin", [C, S], F32R, kind="ExternalInput")
    wqk0 = nc.dram_tensor("wqk0", [C, 2 * 128], F32R, kind="ExternalInput")
    wqk1 = nc.dram_tensor("wqk1", [C, 2 * 128], F32R, kind="ExternalInput")
    wvt = nc.dram_tensor("wvt", [C, O], F32R, kind="ExternalInput")
    wpt = nc.dram_tensor("wpt", [O, O], BF16, kind="ExternalInput")
    biasd = nc.dram_tensor("biasd", [3 * 256], F32, kind="ExternalInput")
    bprowd = nc.dram_tensor("bprow", [1, 256], BF16, kind="ExternalInput")
    # msk2[:, 0:128] = identity, msk2[:, 128:256] = -1e30 * strict_lower
    msk2d = nc.dram_tensor("msk2", [128, 256], BF16, kind="ExternalInput")
    outd = nc.dram_tensor("out", [O, S], F32, kind="ExternalOutput")

    with TileContext(nc) as tc:
        with (
            tc.tile_pool(name="cst", bufs=1) as cst,
            tc.tile_pool(name="ptp", bufs=3) as ptp,
            tc.tile_pool(name="rbp", bufs=4) as rbp,
            tc.tile_pool(name="osb", bufs=4) as osb,
            tc.tile_pool(name="big", bufs=2, space="PSUM") as big,
            tc.tile_pool(name="sml", bufs=1, space="PSUM") as sml,
            tc.tile_pool(name="pap", bufs=2, space="PSUM") as pap,
        ):
            # --- fast start: Pool memsets, ACT table preload, PE clock warm
            wup = cst.tile([128, 64], F32, tag="wup")
            nc.gpsimd.memset(wup, 0.0)
            dmz = cst.tile([128, 1], F32, tag="dmz")
            nc.gpsimd.memset(dmz, 0.0)
            nc.gpsimd.load_library(library_config.attn)
            dme = cst.tile([128, 1], F32, tag="dme")
            nc.scalar.activation(dme, dmz, EXP)
            pwu = big.tile([64, 512], F32, tag="bg")
            for _ in range(N_WARM):
                nc.tensor.matmul(pwu[:, 0:64], wup.bitcast(F32R),
                                 wup[:, 0:64].bitcast(F32R),
                                 start=True, stop=True)

            vaug = cst.tile([128, 8, H, D + 1], BF16, tag="vaug")
            nc.vector.memset(vaug[:, :, :, D], 1.0)

            # --- input DMAs, ordered along the critical path
            wqk = cst.tile([128, NCH, NCH, 2, 128], F32R, tag="wqk")
            wqk_src = [
                d.ap().rearrange("(c p) (t e) -> p c t e", p=128, t=2)
                for d in (wqk0, wqk1)
            ]
            xr = cst.tile([128, NCH, S], F32R, tag="xr")
            xsrc = xin.ap().rearrange("(c p) s -> p c s", p=128)
            bt = cst.tile([128, 3, NCH], F32, tag="bt")
            msk2 = cst.tile([128, 256], BF16, tag="msk2")
            bprow = cst.tile([1, 256], BF16, tag="bprow")
            ones = cst.tile([1, 512], BF16, tag="ones")
            nc.vector.memset(ones, 1.0)
            wv = cst.tile([128, NCH, 256], F32R, tag="wv")
            wp = cst.tile([128, NCH, 256], BF16, tag="wp")

            nc.sync.dma_start(out=wqk[:, :, 0], in_=wqk_src[0])
            nc.sync.dma_start(out=xr[:, :, 0:256], in_=xsrc[:, :, 0:256])
            nc.sync.dma_start(
                out=bt, in_=biasd.ap().rearrange("(b m p) -> p b m", p=128, b=3)
            )
            nc.sync.dma_start(out=msk2, in_=msk2d.ap())
            nc.sync.dma_start(out=bprow, in_=bprowd.ap())
            nc.sync.dma_start(out=xr[:, :, 256:512], in_=xsrc[:, :, 256:512])
            nc.sync.dma_start(out=xr[:, :, 512:1024], in_=xsrc[:, :, 512:1024])
            nc.sync.dma_start(out=wqk[:, :, 1], in_=wqk_src[1])
            nc.sync.dma_start(out=wv, in_=wvt.ap().rearrange("(c p) e -> p c e", p=128))
            nc.sync.dma_start(out=wp, in_=wpt.ap().rearrange("(c p) e -> p c e", p=128))

            ident = msk2[:, 0:128]
            mlow = msk2[:, 128:256]

            q_sb = cst.tile([128, NCH, S], F32R, tag="q_sb")
            k_sb = cst.tile([128, NCH, S], F32R, tag="k_sb")
            att = cst.tile([128, NCH, S], BF16, tag="att")

            def qk_proj(t, m, c0, c1, eng):
                # t: 0 = q, 1 = k; columns [c0, c1); eng: 'dve' | 'act'
                # psum from the pa ring (same 2KB/partition slot size), which
                # is otherwise idle while projections run
                dst = (q_sb, k_sb)[t]
                pp = pap.tile([128, 512], F32, tag="pa")
                w = c1 - c0
                for c in range(2):
                    nc.tensor.matmul(
                        pp[:, 0:w],
                        wqk[:, c, m, t, :],
                        xr[:, c, c0:c1],
                        start=(c == 0), stop=(c == 1),
                    )
                if eng == 'act':
                    nc.scalar.add(dst[:, m, c0:c1], pp[:, 0:w], bt[:, t, m:m + 1])
                else:
                    nc.vector.tensor_scalar_add(
                        dst[:, m, c0:c1], pp[:, 0:w], bt[:, t, m:m + 1]
                    )

            # pts is PACKED: chunk i's live columns [128i, S) stored at
            # [POFF[i], POFF[i+1]), so merged exps write contiguous ranges
            POFF = [0, 1024, 1920, 2688, 3328, 3840, 4224, 4480, 4608]
            pts_tiles = {}

            def get_pts(h):
                if h not in pts_tiles:
                    pts = ptp.tile([128, POFF[8]], BF16, tag="pts",
                                   name=f"pts{h}")
                    pts_tiles[h] = pts
                return pts_tiles[h]

            def pcol(i, c):
                # pts column for chunk i, sq position c
                return POFF[i] + c - 128 * i

            def sc_win(h, i, ps, off, j):
                # score matmuls for sq window j of sk chunk i (+ PE mask on
                # the diagonal block window)
                m, r = h // 4, h % 4
                rows = slice(32 * r, 32 * r + 32)
                we = 512 * (j + 1)
                if we <= 128 * i:
                    return
                ws = max(512 * j, 128 * i)
                ws_mm = max(min(ws, we - 256), 512 * j)
                diag = ws == 128 * i  # window containing the diagonal block
                nc.tensor.matmul(
                    ps[:, ws_mm - off:we - off],
                    k_sb[rows, m, 128 * i:128 * (i + 1)],
                    q_sb[rows, m, ws_mm:we],
                    start=True, stop=not diag,
                    tile_position=(32 * r, 0),
                )
                if diag:
                    # psum[sk, sq] += -1e30 for sq < sk within the block
                    nc.tensor.matmul(
                        ps[:, 128 * i - off:128 * (i + 1) - off],
                        ident, mlow,
                        start=False, stop=True,
                    )

            def sc_exp(h, i, ps, off, e0, e1):
                nc.scalar.activation(
                    get_pts(h)[:, pcol(i, e0):pcol(i, e1)],
                    ps[:, e0 - off:e1 - off], EXP
                )

            def sc(h, i):
                # steady-state chunk 0-3: both windows + one exp (big ring)
                get_pts(h)
                ps = big.tile([128, S], F32, tag="bg", name=f"ps{h}_{i}")
                for j in range(2):
                    sc_win(h, i, ps, 0, j)
                sc_exp(h, i, ps, 0, 128 * i, S)

            def sc45(h):
                # chunks 4+5 share a big tile (c4 at [0:512], c5 at
                # [512:896]) and ONE exp over the packed pts range
                m, r = h // 4, h % 4
                rows = slice(32 * r, 32 * r + 32)
                ps = big.tile([128, S], F32, tag="bg", name=f"ps{h}_45")
                nc.tensor.matmul(ps[:, 0:512], k_sb[rows, m, 512:640],
                                 q_sb[rows, m, 512:1024], start=True,
                                 stop=False, tile_position=(32 * r, 0))
                nc.tensor.matmul(ps[:, 0:128], ident, mlow,
                                 start=False, stop=True)
                nc.tensor.matmul(ps[:, 512:896], k_sb[rows, m, 640:768],
                                 q_sb[rows, m, 640:1024], start=True,
                                 stop=False, tile_position=(32 * r, 0))
                nc.tensor.matmul(ps[:, 512:640], ident, mlow,
                                 start=False, stop=True)
                nc.scalar.activation(
                    get_pts(h)[:, POFF[4]:POFF[6]], ps[:, 0:896], EXP)

            def sc67(h):
                # chunks 6+7 share a small tile (c6 at [0:256], c7 at
                # [256:384], unwidened) and ONE exp
                m, r = h // 4, h % 4
                rows = slice(32 * r, 32 * r + 32)
                ps = sml.tile([128, 512], F32, tag="c67", name=f"ps{h}_67")
                nc.tensor.matmul(ps[:, 0:256], k_sb[rows, m, 768:896],
                                 q_sb[rows, m, 768:1024], start=True,
                                 stop=False, tile_position=(32 * r, 0))
                nc.tensor.matmul(ps[:, 0:128], ident, mlow,
                                 start=False, stop=True)
                nc.tensor.matmul(ps[:, 256:384], k_sb[rows, m, 896:1024],
                                 q_sb[rows, m, 896:1024], start=True,
                                 stop=False, tile_position=(32 * r, 0))
                nc.tensor.matmul(ps[:, 256:384], ident, mlow,
                                 start=False, stop=True)
                nc.scalar.activation(
                    get_pts(h)[:, POFF[6]:POFF[8]], ps[:, 0:384], EXP)

            def att_mms(h, pa, ii, q0, q1, first, last, base=0):
                # accumulate sq columns [q0, q1) into pa[:, q0-base:q1-base]
                pts = pts_tiles[h]
                for idx, i in enumerate(ii):
                    ws = max(q0, 128 * i)
                    nc.tensor.matmul(
                        pa[:, ws - base:q1 - base],
                        vaug[:, i, h, :],
                        pts[:, pcol(i, ws):pcol(i, q1)],
                        start=(first and idx == 0),
                        stop=(last and idx == len(ii) - 1),
                    )

            def att_recip_bcast(pa, p0, p1):
                w = p1 - p0
                rf = rbp.tile([1, 512], F32, tag="rf")
                nc.vector.reciprocal(rf[:, 0:w], pa[32:33, p0:p1])
                rb = rbp.tile([32, 512], F32, tag="rb")
                nc.gpsimd.partition_broadcast(rb[:, 0:w], rf[:, 0:w])
                return rb

            def att_mul(h, pa, rb, q0, q1, p0=0):
                # att columns [q0, q1) normalized from pa[0:32, p0:p0+w]
                m, r = h // 4, h % 4
                w = q1 - q0
                nc.vector.tensor_mul(
                    att[32 * r:32 * r + 32, m, q0:q1],
                    pa[0:32, p0:p0 + w], rb[:, 0:w],
                )

            out_ap = outd.ap().rearrange("(m p) s -> p m s", p=128)

            def po_piece(q0, q1, eng):
                # output projection for columns [q0, q1); the bias is folded
                # in as a bias-row x ones accumulate matmul, so the psum ->
                # SBUF move is a single plain copy per piece
                w = q1 - q0
                po = big.tile([128, 2, 512], F32, tag="bg", name=f"po{q0}")
                for m in range(2):
                    for c in range(2):
                        nc.tensor.matmul(
                            po[:, m, 0:w],
                            wp[:, c, m * 128:(m + 1) * 128],
                            att[:, c, q0:q1],
                            start=(c == 0), stop=False,
                        )
                    nc.tensor.matmul(
                        po[:, m, 0:w],
                        bprow[:, m * 128:(m + 1) * 128],
                        ones[:, 0:w],
                        start=False, stop=True,
                    )
                ot = osb.tile([128, 2, 512], F32, tag="ot", name=f"ot{q0}")
                if eng == 'act':
                    nc.scalar.copy(ot[:, :, 0:w], po[:, :, 0:w])
                else:
                    nc.vector.tensor_copy(ot[:, :, 0:w], po[:, :, 0:w])
                nc.sync.dma_start(out=out_ap[:, :, q0:q1], in_=ot[:, :, 0:w])

            # ---------------- head 0 priming ----------------
            def sc_win0(i, ps, off, w0, w1):
                # head-0 score mms + exp for sq window [w0, w1) of chunk i
                ws = max(w0, 128 * i)
                if w1 <= ws:
                    return
                ws_mm = max(min(ws, w1 - 256), w0)
                diag = ws == 128 * i
                nc.tensor.matmul(
                    ps[:, ws_mm - off:w1 - off],
                    k_sb[0:32, 0, 128 * i:128 * (i + 1)],
                    q_sb[0:32, 0, ws_mm:w1],
                    start=True, stop=not diag,
                    tile_position=(0, 0),
                )
                if diag:
                    nc.tensor.matmul(
                        ps[:, 128 * i - off:128 * (i + 1) - off],
                        ident, mlow,
                        start=False, stop=True,
                    )
                nc.scalar.activation(
                    get_pts(0)[:, pcol(i, ws):pcol(i, w1)],
                    ps[:, ws - off:w1 - off], EXP
                )

            qk_proj(0, 0, 0, 256, 'dve')
            qk_proj(1, 0, 0, 256, 'act')
            b0 = big.tile([128, S], F32, tag="bg", name="ps0_0")
            b1 = big.tile([128, S], F32, tag="bg", name="ps0_1")
            sc_win0(0, b0, 0, 0, 256)
            sc_win0(1, b1, 0, 0, 256)
            qk_proj(0, 0, 256, 512, 'dve')
            qk_proj(1, 0, 256, 512, 'act')
            sc_win0(0, b0, 0, 256, 512)
            sc_win0(1, b1, 0, 256, 512)
            # chunks 2/3's j0 windows share one small tile (disjoint cols)
            s23a = sml.tile([128, 512], F32, tag="sm", name="ps0_23a")
            sc_win0(2, s23a, 256, 256, 512)
            sc_win0(3, s23a, 0, 256, 512)
            # j1 windows (x second half); chunks 2/3 share one big tile
            qk_proj(0, 0, 512, 1024, 'dve')
            qk_proj(1, 0, 512, 1024, 'act')
            sc_win0(0, b0, 0, 512, 1024)
            sc_win0(1, b1, 0, 512, 1024)
            s23b = big.tile([128, S], F32, tag="bg", name="ps0_23b")
            sc_win0(2, s23b, 512, 512, 1024)
            sc_win0(3, s23b, 0, 512, 1024)
            sc45(0)
            sc(1, 0)
            sc67(0)

            # v projection (after wv lands) + head-0 attnv j0
            def v_proj(i):
                pv = pap.tile([128, 512], F32, tag="pa", name=f"pv{i}")
                for c in range(2):
                    nc.tensor.matmul(
                        pv[:, 0:256],
                        xr[:, c, i * 128:(i + 1) * 128],
                        wv[:, c, :],
                        start=(c == 0), stop=(c == 1),
                    )
                nc.vector.tensor_copy(
                    vaug[:, i, :, 0:D],
                    pv[:, 0:256].rearrange("p (h d) -> p h d", h=H),
                )

            # m1 q/k projections: wqk1 lands mid-head-0; doing these here
            # keeps their psum slots and bias-adds off the steady-state path
            qk_proj(0, 1, 0, 512, 'dve')
            qk_proj(1, 1, 0, 512, 'dve')
            qk_proj(0, 1, 512, 1024, 'dve')
            qk_proj(1, 1, 512, 1024, 'dve')

            for i in range(8):
                v_proj(i)

            # One [33,512] accumulator per head: j0 round in head h, then the
            # j1 round REUSES the same tile in h+1 (after the j0 multiply)
            pa_att = {}  # h -> accumulator tile
            pa_j0 = {}   # h -> rb for the j0 half

            def attn_j0(h):
                pa = pap.tile([33, 512], F32, tag="pa", name=f"pa{h}")
                pa_att[h] = pa
                att_mms(h, pa, [0, 1, 2, 3], 0, 512, True, True)
                pa_j0[h] = att_recip_bcast(pa, 0, 512)

            attn_j0(0)
            att_mul(0, pa_att[0], pa_j0.pop(0), 0, 512)

            # ---------------- steady heads ----------------
            def emit_steady(h):
                prev = h - 1
                for i in (1, 2, 3):
                    sc(h, i)
                sc45(h)
                # attnv j0 of h: mms ready after e3; PE reaches them here
                attn_j0(h)
                sc(h + 1, 0)
                sc67(h)
                # j0 multiply of h (dep: bcast just emitted)
                att_mul(h, pa_att[h], pa_j0.pop(h), 0, 512)
                # attnv j1 of prev, reusing its accumulator (j0 mul done)
                pa_prev = pa_att.pop(prev)
                att_mms(prev, pa_prev, [0, 1, 2, 3], 512, 1024, True, False,
                        base=512)
                att_mms(prev, pa_prev, [4, 5, 6, 7], 512, 1024, False, True,
                        base=512)
                rbj1 = att_recip_bcast(pa_prev, 0, 512)
                att_mul(prev, pa_prev, rbj1, 512, 1024, p0=0)
                pts_tiles.pop(prev)

            for h in range(1, 7):
                emit_steady(h)

            # ---------------- head 7 + tail ----------------
            p7 = get_pts(7)
            pa6 = pa_att.pop(6)
            sc(7, 1)
            sc(7, 2)
            # j1 of head 6, interleaved with head-7 scores
            att_mms(6, pa6, [0, 1], 512, 1024, True, False, base=512)
            sc(7, 3)
            att_mms(6, pa6, [2, 3], 512, 1024, False, False, base=512)
            sc45(7)
            # attnv j0 of head 7, group A: cols [0:256) needs chunks 0,1
            pa7 = pap.tile([33, 512], F32, tag="pa", name="pa7")
            nc.tensor.matmul(pa7[:, 0:256], vaug[:, 0, 7, :], p7[:, 0:256],
                             start=True, stop=False)
            nc.tensor.matmul(pa7[:, 128:256], vaug[:, 1, 7, :],
                             p7[:, pcol(1, 128):pcol(1, 256)],
                             start=False, stop=True)
            rb_a = att_recip_bcast(pa7, 0, 256)
            att_mms(6, pa6, [4, 5, 6, 7], 512, 1024, False, True, base=512)
            rbj1_6 = att_recip_bcast(pa6, 0, 512)
            # group B: cols [256:512) needs chunks 0-3
            nc.tensor.matmul(pa7[:, 256:512], vaug[:, 0, 7, :], p7[:, 256:512],
                             start=True, stop=False)
            nc.tensor.matmul(pa7[:, 256:512], vaug[:, 1, 7, :],
                             p7[:, pcol(1, 256):pcol(1, 512)],
                             start=False, stop=False)
            nc.tensor.matmul(pa7[:, 256:512], vaug[:, 2, 7, :],
                             p7[:, pcol(2, 256):pcol(2, 512)],
                             start=False, stop=False)
            nc.tensor.matmul(pa7[:, 384:512], vaug[:, 3, 7, :],
                             p7[:, pcol(3, 384):pcol(3, 512)],
                             start=False, stop=True)
            rb_b = att_recip_bcast(pa7, 256, 512)
            sc67(7)
            pts_tiles.pop(6)
            # tail accumulators: [512:768) at cols [0:256) and [768:1024)
            # at cols [256:512) of one shared small-ring tile
            pa_cde = sml.tile([33, 512], F32, tag="sm", name="pa_cde")
            att_mms(7, pa_cde, [0, 1, 2, 3, 4, 5], 512, 768, True, True,
                    base=512)
            att_mms(7, pa_cde, [0, 1, 2, 3, 4, 5], 768, 1024, True, False,
                    base=512)
            # muls + outproj pieces, pipelined against the exp tail
            att_mul(7, pa7, rb_a, 0, 256)
            po_piece(0, 256, 'act')
            rb_c = att_recip_bcast(pa_cde, 0, 256)
            att_mul(6, pa6, rbj1_6, 512, 1024, p0=0)
            att_mul(7, pa7, rb_b, 256, 512, p0=256)
            po_piece(256, 512, 'act')
            att_mms(7, pa_cde, [6, 7], 768, 1024, False, True, base=512)
            rb_de = att_recip_bcast(pa_cde, 256, 512)
            att_mul(7, pa_cde, rb_c, 512, 768, p0=0)
            po_piece(512, 768, 'act')
            att_mul(7, pa_cde, rb_de, 768, 1024, p0=256)
            po_piece(768, 1024, 'act')

    nc.compile()
    return nc


def get_program():
    if "nc" not in _CACHE:
        _CACHE["nc"] = _build_program()
    return _CACHE["nc"]


def kernel(x, wq, bq, wkv, bkv, wproj, bproj):
    import ml_dtypes
    from concourse.bass_utils import run_bass_kernel_spmd

    nc = get_program()

    x = np.asarray(x, dtype=np.float32)
    n = x.shape[0]
    assert n == N_CORES and x.shape[1:] == (C, 32, 32)

    scale = 1.0 / np.sqrt(np.float32(D))
    wq_s = np.asarray(wq, np.float32) * scale
    bq_s = np.asarray(bq, np.float32) * scale
    wk = np.asarray(wkv[:E], np.float32)
    bk = np.asarray(bkv[:E], np.float32)
    wv = np.asarray(wkv[E:], np.float32)
    bv = np.asarray(bkv[E:], np.float32)
    wproj = np.asarray(wproj, np.float32)
    bproj_eff = (np.asarray(bproj, np.float32)
                 + wproj.astype(np.float64) @ bv.astype(np.float64)).astype(np.float32)

    # msk2: [identity | -1e30 * strict_lower(r > sq)]
    ident = np.eye(128, dtype=np.float32)
    mlow = np.where(np.arange(128)[:, None] > np.arange(128)[None, :],
                    np.float32(-1e30), np.float32(0.0))
    msk2 = np.concatenate([ident, mlow], axis=1).astype(ml_dtypes.bfloat16)

    shared = {
        "wqk0": np.ascontiguousarray(
            np.concatenate([wq_s.T[:, 0:128], wk.T[:, 0:128]], axis=1)),
        "wqk1": np.ascontiguousarray(
            np.concatenate([wq_s.T[:, 128:256], wk.T[:, 128:256]], axis=1)),
        "wvt": np.ascontiguousarray(wv.T),
        "wpt": np.ascontiguousarray(wproj.T.astype(ml_dtypes.bfloat16)),
        "biasd": np.ascontiguousarray(
            np.concatenate([bq_s, bk, bproj_eff])),
        "bprow": np.ascontiguousarray(
            bproj_eff.reshape(1, 256).astype(ml_dtypes.bfloat16)),
        "msk2": np.ascontiguousarray(msk2),
    }
    in_maps = [
        {"xin": np.ascontiguousarray(x[i].reshape(C, S)), **shared}
        for i in range(N_CORES)
    ]
    res = run_bass_kernel_spmd(nc, in_maps, core_ids=list(range(N_CORES)))
    out = np.stack([res.results[i]["out"].reshape(O, 32, 32) for i in range(N_CORES)])
    return out.astype(np.float32)


# revision 60
# speedup vs baseline: 1.1627x; 1.0107x over previous
"""Causal multi-head attention (8 heads, 1x1-conv projections) on 8 TRN2 cores.

Sharding: data-parallel over batch N=8 -> one batch element per NeuronCore.
Per-core kernel (S=1024 pixels, C=E=256 channels, H=8 heads, d=32):
  q = WqT.T @ x, k = WkT.T @ x              (e, s) layout, fp32r matmuls
  vT = x.T @ WvT                            (s, e) layout
  per head: P^T[sk, sq] = exp(mask(k_h^T q_h))   scores computed TRANSPOSED so
                                            softmax denominator comes from an
                                            appended ones-column in v (M=33)
  out_h = (vAug_h^T @ P^T) -> rows 0..31 numerator^T, row 32 = denominator
  att = num / denom (bf16), out = WprojT.T @ att + bproj_eff

Design (v2, ACT-stream-paced):
- The exp stream on the Activation engine is the pacing resource (~44us of
  exp work).  Everything else is scheduled around keeping it gap-free.
- Causal masking is folded into the score matmuls: an extra bf16 accumulate
  matmul (identity stationary x strictly-lower -BIG moving) writes -1e30
  into the masked diagonal-block region of PSUM before the exp, so exp
  emits exact zeros and the DVE never touches masks.
- Per head h: attnv j0-half accumulates during head h (after exp chunk 3),
  the j1-half during head h+1.  Softmax normalization: DVE reciprocal of
  the denominator row, gpsimd partition_broadcast, deferred DVE multiply.
- exp emission order per head: e1..e5, e0(next head), e6, e7 -- psum-ring
  write-after-read hazards then never block a score matmul near its exp.
- PSUM: 2x[128,1024] ring (chunks 0-3 + outproj), 2x[128,512] ring
  (chunks 4-7 + q/k/v projections), 2x[33,512] ring (attnv accumulators).
- Output projection + DMA run in 4 column pieces ([0:256], [256:512],
  [512:768], [768:1024]) pipelined against the last head's exp tail; att
  and Wproj are bf16 so narrow matmuls run at full PE rate.
- Startup: x[:, 0:512] lands early (one DMA queue, transfers serialize),
  PE warmup matmuls start at ~250ns so the PE clock is ramped before the
  first projection; head 0 primes the stream with j0-window exps.
"""

import numpy as np

N_CORES = 8
C = 256      # input channels
E = 256      # embed channels (q/k)
O = 256      # v/out channels
S = 1024     # spatial positions (32*32)
H = 8        # heads
D = 32       # head dim
NCH = 2      # 256 = 2 * 128 partition chunks
N_WARM = 33  # PE clock-warm matmuls

_CACHE = {}


def _build_program():
    import concourse.mybir as mybir
    from concourse import bacc
    from concourse import library_config
    from concourse.tile import TileContext

    F32 = mybir.dt.float32
    F32R = mybir.dt.float32r
    BF16 = mybir.dt.bfloat16
    EXP = mybir.ActivationFunctionType.Exp

    nc = bacc.Bacc("TRN2", target_bir_lowering=False, debug=False)

    xin = nc.dram_tensor("xin", [C, S], F32R, kind="ExternalInput")
    wqk0 = nc.dram_tensor("wqk0", [C, 2 * 128], F32R, kind="ExternalInput")
    wqk1 = nc.dram_tensor("wqk1", [C, 2 * 128], F32R, kind="ExternalInput")
    wvt = nc.dram_tensor("wvt", [C, O], F32R, kind="ExternalInput")
    wpt = nc.dram_tensor("wpt", [O, O], BF16, kind="ExternalInput")
    biasd = nc.dram_tensor("biasd", [3 * 256], F32, kind="ExternalInput")
    bprowd = nc.dram_tensor("bprow", [1, 256], BF16, kind="ExternalInput")
    # msk2[:, 0:128] = identity, msk2[:, 128:256] = -1e30 * strict_lower
    msk2d = nc.dram_tensor("msk2", [128, 256], BF16, kind="ExternalInput")
    outd = nc.dram_tensor("out", [O, S], F32, kind="ExternalOutput")

    with TileContext(nc) as tc:
        with (
            tc.tile_pool(name="cst", bufs=1) as cst,
            tc.tile_pool(name="ptp", bufs=3) as ptp,
            tc.tile_pool(name="rbp", bufs=4) as rbp,
            tc.tile_pool(name="osb", bufs=4) as osb,
            tc.tile_pool(name="big", bufs=2, space="PSUM") as big,
            tc.tile_pool(name="sml", bufs=1, space="PSUM") as sml,
            tc.tile_pool(name="pap", bufs=2, space="PSUM") as pap,
        ):
            # --- fast start: Pool memsets, ACT table preload, PE clock warm
            wup = cst.tile([128, 64], F32, tag="wup")
            nc.gpsimd.memset(wup, 0.0)
            dmz = cst.tile([128, 1], F32, tag="dmz")
            nc.gpsimd.memset(dmz, 0.0)
            nc.gpsimd.load_library(library_config.attn)
            dme = cst.tile([128, 1], F32, tag="dme")
            nc.scalar.activation(dme, dmz, EXP)
            pwu = big.tile([64, 512], F32, tag="bg")
            for _ in range(N_WARM):
                nc.tensor.matmul(pwu[:, 0:64], wup.bitcast(F32R),
                                 wup[:, 0:64].bitcast(F32R),
                                 start=True, stop=True)

            vaug = cst.tile([128, 8, H, D + 1], BF16, tag="vaug")
            nc.vector.memset(vaug[:, :, :, D], 1.0)

            # --- input DMAs, ordered along the critical path
            wqk = cst.tile([128, NCH, NCH, 2, 128], F32R, tag="wqk")
            wqk_src = [
                d.ap().rearrange("(c p) (t e) -> p c t e", p=128, t=2)
                for d in (wqk0, wqk1)
            ]
            xr = cst.tile([128, NCH, S], F32R, tag="xr")
            xsrc = xin.ap().rearrange("(c p) s -> p c s", p=128)
            bt = cst.tile([128, 3, NCH], F32, tag="bt")
            msk2 = cst.tile([128, 256], BF16, tag="msk2")
            bprow = cst.tile([1, 256], BF16, tag="bprow")
            ones = cst.tile([1, 512], BF16, tag="ones")
            nc.vector.memset(ones, 1.0)
            wv = cst.tile([128, NCH, 256], F32R, tag="wv")
            wp = cst.tile([128, NCH, 256], BF16, tag="wp")

            nc.sync.dma_start(out=wqk[:, :, 0], in_=wqk_src[0])
            nc.sync.dma_start(out=xr[:, :, 0:256], in_=xsrc[:, :, 0:256])
            nc.sync.dma_start(
                out=bt, in_=biasd.ap().rearrange("(b m p) -> p b m", p=128, b=3)
            )
            nc.sync.dma_start(out=msk2, in_=msk2d.ap())
            nc.sync.dma_start(out=xr[:, :, 256:512], in_=xsrc[:, :, 256:512])
            nc.sync.dma_start(out=xr[:, :, 512:1024], in_=xsrc[:, :, 512:1024])
            nc.sync.dma_start(out=wqk[:, :, 1], in_=wqk_src[1])
            nc.sync.dma_start(out=wv, in_=wvt.ap().rearrange("(c p) e -> p c e", p=128))
            nc.sync.dma_start(out=wp, in_=wpt.ap().rearrange("(c p) e -> p c e", p=128))
            nc.sync.dma_start(out=bprow, in_=bprowd.ap())

            ident = msk2[:, 0:128]
            mlow = msk2[:, 128:256]

            q_sb = cst.tile([128, NCH, S], F32R, tag="q_sb")
            k_sb = cst.tile([128, NCH, S], F32R, tag="k_sb")
            att = cst.tile([128, NCH, S], BF16, tag="att")

            def qk_proj(t, m, c0, c1, eng):
                # t: 0 = q, 1 = k; columns [c0, c1); eng: 'dve' | 'act'
                # psum from the pa ring (same 2KB/partition slot size), which
                # is otherwise idle while projections run
                dst = (q_sb, k_sb)[t]
                pp = pap.tile([128, 512], F32, tag="pa")
                w = c1 - c0
                for c in range(2):
                    nc.tensor.matmul(
                        pp[:, 0:w],
                        wqk[:, c, m, t, :],
                        xr[:, c, c0:c1],
                        start=(c == 0), stop=(c == 1),
                    )
                if eng == 'act':
                    nc.scalar.add(dst[:, m, c0:c1], pp[:, 0:w], bt[:, t, m:m + 1])
                else:
                    nc.vector.tensor_scalar_add(
                        dst[:, m, c0:c1], pp[:, 0:w], bt[:, t, m:m + 1]
                    )

            # pts is PACKED: chunk i's live columns [128i, S) stored at
            # [POFF[i], POFF[i+1]), so merged exps write contiguous ranges
            POFF = [0, 1024, 1920, 2688, 3328, 3840, 4224, 4480, 4608]
            pts_tiles = {}

            def get_pts(h):
                if h not in pts_tiles:
                    pts = ptp.tile([128, POFF[8]], BF16, tag="pts",
                                   name=f"pts{h}")
                    pts_tiles[h] = pts
                return pts_tiles[h]

            def pcol(i, c):
                # pts column for chunk i, sq position c
                return POFF[i] + c - 128 * i

            def sc_win(h, i, ps, off, j):
                # score matmuls for sq window j of sk chunk i (+ PE mask on
                # the diagonal block window)
                m, r = h // 4, h % 4
                rows = slice(32 * r, 32 * r + 32)
                we = 512 * (j + 1)
                if we <= 128 * i:
                    return
                ws = max(512 * j, 128 * i)
                ws_mm = max(min(ws, we - 256), 512 * j)
                diag = ws == 128 * i  # window containing the diagonal block
                nc.tensor.matmul(
                    ps[:, ws_mm - off:we - off],
                    k_sb[rows, m, 128 * i:128 * (i + 1)],
                    q_sb[rows, m, ws_mm:we],
                    start=True, stop=not diag,
                    tile_position=(32 * r, 0),
                )
                if diag:
                    # psum[sk, sq] += -1e30 for sq < sk within the block
                    nc.tensor.matmul(
                        ps[:, 128 * i - off:128 * (i + 1) - off],
                        ident, mlow,
                        start=False, stop=True,
                    )

            def sc_exp(h, i, ps, off, e0, e1):
                nc.scalar.activation(
                    get_pts(h)[:, pcol(i, e0):pcol(i, e1)],
                    ps[:, e0 - off:e1 - off], EXP
                )

            def sc(h, i):
                # steady-state chunk 0-3: both windows + one exp (big ring)
                get_pts(h)
                ps = big.tile([128, S], F32, tag="bg", name=f"ps{h}_{i}")
                for j in range(2):
                    sc_win(h, i, ps, 0, j)
                sc_exp(h, i, ps, 0, 128 * i, S)

            def sc45(h):
                # chunks 4+5 share a big tile (c4 at [0:512], c5 at
                # [512:896]) and ONE exp over the packed pts range
                m, r = h // 4, h % 4
                rows = slice(32 * r, 32 * r + 32)
                ps = big.tile([128, S], F32, tag="bg", name=f"ps{h}_45")
                nc.tensor.matmul(ps[:, 0:512], k_sb[rows, m, 512:640],
                                 q_sb[rows, m, 512:1024], start=True,
                                 stop=False, tile_position=(32 * r, 0))
                nc.tensor.matmul(ps[:, 0:128], ident, mlow,
                                 start=False, stop=True)
                nc.tensor.matmul(ps[:, 512:896], k_sb[rows, m, 640:768],
                                 q_sb[rows, m, 640:1024], start=True,
                                 stop=False, tile_position=(32 * r, 0))
                nc.tensor.matmul(ps[:, 512:640], ident, mlow,
                                 start=False, stop=True)
                nc.scalar.activation(
                    get_pts(h)[:, POFF[4]:POFF[6]], ps[:, 0:896], EXP)

            def sc67(h):
                # chunks 6+7 share a small tile (c6 at [0:256], c7 at
                # [256:384], unwidened) and ONE exp
                m, r = h // 4, h % 4
                rows = slice(32 * r, 32 * r + 32)
                ps = sml.tile([128, 512], F32, tag="c67", name=f"ps{h}_67")
                nc.tensor.matmul(ps[:, 0:256], k_sb[rows, m, 768:896],
                                 q_sb[rows, m, 768:1024], start=True,
                                 stop=False, tile_position=(32 * r, 0))
                nc.tensor.matmul(ps[:, 0:128], ident, mlow,
                                 start=False, stop=True)
                nc.tensor.matmul(ps[:, 256:384], k_sb[rows, m, 896:1024],
                                 q_sb[rows, m, 896:1024], start=True,
                                 stop=False, tile_position=(32 * r, 0))
                nc.tensor.matmul(ps[:, 256:384], ident, mlow,
                                 start=False, stop=True)
                nc.scalar.activation(
                    get_pts(h)[:, POFF[6]:POFF[8]], ps[:, 0:384], EXP)

            def att_mms(h, pa, ii, q0, q1, first, last, base=0):
                # accumulate sq columns [q0, q1) into pa[:, q0-base:q1-base]
                pts = pts_tiles[h]
                for idx, i in enumerate(ii):
                    ws = max(q0, 128 * i)
                    nc.tensor.matmul(
                        pa[:, ws - base:q1 - base],
                        vaug[:, i, h, :],
                        pts[:, pcol(i, ws):pcol(i, q1)],
                        start=(first and idx == 0),
                        stop=(last and idx == len(ii) - 1),
                    )

            def att_recip_bcast(pa, p0, p1):
                w = p1 - p0
                rf = rbp.tile([1, 512], F32, tag="rf")
                nc.vector.reciprocal(rf[:, 0:w], pa[32:33, p0:p1])
                rb = rbp.tile([32, 512], F32, tag="rb")
                nc.gpsimd.partition_broadcast(rb[:, 0:w], rf[:, 0:w])
                return rb

            def att_mul(h, pa, rb, q0, q1, p0=0):
                # att columns [q0, q1) normalized from pa[0:32, p0:p0+w]
                m, r = h // 4, h % 4
                w = q1 - q0
                nc.vector.tensor_mul(
                    att[32 * r:32 * r + 32, m, q0:q1],
                    pa[0:32, p0:p0 + w], rb[:, 0:w],
                )

            out_ap = outd.ap().rearrange("(m p) s -> p m s", p=128)

            def po_piece(q0, q1, eng):
                # output projection for columns [q0, q1); the bias is folded
                # in as a bias-row x ones accumulate matmul, so the psum ->
                # SBUF move is a single plain copy per piece
                w = q1 - q0
                po = big.tile([128, 2, 512], F32, tag="bg", name=f"po{q0}")
                for m in range(2):
                    for c in range(2):
                        nc.tensor.matmul(
                            po[:, m, 0:w],
                            wp[:, c, m * 128:(m + 1) * 128],
                            att[:, c, q0:q1],
                            start=(c == 0), stop=False,
                        )
                    nc.tensor.matmul(
                        po[:, m, 0:w],
                        bprow[:, m * 128:(m + 1) * 128],
                        ones[:, 0:w],
                        start=False, stop=True,
                    )
                ot = osb.tile([128, 2, 512], F32, tag="ot", name=f"ot{q0}")
                if eng == 'act':
                    nc.scalar.copy(ot[:, :, 0:w], po[:, :, 0:w])
                else:
                    nc.vector.tensor_copy(ot[:, :, 0:w], po[:, :, 0:w])
                nc.sync.dma_start(out=out_ap[:, :, q0:q1], in_=ot[:, :, 0:w])

            # ---------------- head 0 priming ----------------
            def sc_win0(i, ps, off, w0, w1):
                # head-0 score mms + exp for sq window [w0, w1) of chunk i
                ws = max(w0, 128 * i)
                if w1 <= ws:
                    return
                ws_mm = max(min(ws, w1 - 256), w0)
                diag = ws == 128 * i
                nc.tensor.matmul(
                    ps[:, ws_mm - off:w1 - off],
                    k_sb[0:32, 0, 128 * i:128 * (i + 1)],
                    q_sb[0:32, 0, ws_mm:w1],
                    start=True, stop=not diag,
                    tile_position=(0, 0),
                )
                if diag:
                    nc.tensor.matmul(
                        ps[:, 128 * i - off:128 * (i + 1) - off],
                        ident, mlow,
                        start=False, stop=True,
                    )
                nc.scalar.activation(
                    get_pts(0)[:, pcol(i, ws):pcol(i, w1)],
                    ps[:, ws - off:w1 - off], EXP
                )

            qk_proj(0, 0, 0, 256, 'dve')
            qk_proj(1, 0, 0, 256, 'act')
            b0 = big.tile([128, S], F32, tag="bg", name="ps0_0")
            b1 = big.tile([128, S], F32, tag="bg", name="ps0_1")
            sc_win0(0, b0, 0, 0, 256)
            sc_win0(1, b1, 0, 0, 256)
            qk_proj(0, 0, 256, 512, 'dve')
            qk_proj(1, 0, 256, 512, 'act')
            sc_win0(0, b0, 0, 256, 512)
            sc_win0(1, b1, 0, 256, 512)
            # chunks 2/3's j0 windows share one small tile (disjoint cols)
            s23a = sml.tile([128, 512], F32, tag="sm", name="ps0_23a")
            sc_win0(2, s23a, 256, 256, 512)
            sc_win0(3, s23a, 0, 256, 512)
            # j1 windows (x second half); chunks 2/3 share one big tile
            qk_proj(0, 0, 512, 1024, 'dve')
            qk_proj(1, 0, 512, 1024, 'act')
            sc_win0(0, b0, 0, 512, 1024)
            sc_win0(1, b1, 0, 512, 1024)
            s23b = big.tile([128, S], F32, tag="bg", name="ps0_23b")
            sc_win0(2, s23b, 512, 512, 1024)
            sc_win0(3, s23b, 0, 512, 1024)
            sc45(0)
            sc(1, 0)
            sc67(0)

            # v projection (after wv lands) + head-0 attnv j0
            def v_proj(i):
                pv = pap.tile([128, 512], F32, tag="pa", name=f"pv{i}")
                for c in range(2):
                    nc.tensor.matmul(
                        pv[:, 0:256],
                        xr[:, c, i * 128:(i + 1) * 128],
                        wv[:, c, :],
                        start=(c == 0), stop=(c == 1),
                    )
                nc.vector.tensor_copy(
                    vaug[:, i, :, 0:D],
                    pv[:, 0:256].rearrange("p (h d) -> p h d", h=H),
                )

            # m1 q/k projections: wqk1 lands mid-head-0; doing these here
            # keeps their psum slots and bias-adds off the steady-state path
            qk_proj(0, 1, 0, 512, 'dve')
            qk_proj(1, 1, 0, 512, 'dve')
            qk_proj(0, 1, 512, 1024, 'dve')
            qk_proj(1, 1, 512, 1024, 'dve')

            for i in range(8):
                v_proj(i)

            # One [33,512] accumulator per head: j0 round in head h, then the
            # j1 round REUSES the same tile in h+1 (after the j0 multiply)
            pa_att = {}  # h -> accumulator tile
            pa_j0 = {}   # h -> rb for the j0 half

            def attn_j0(h):
                pa = pap.tile([33, 512], F32, tag="pa", name=f"pa{h}")
                pa_att[h] = pa
                att_mms(h, pa, [0, 1, 2, 3], 0, 512, True, True)
                pa_j0[h] = att_recip_bcast(pa, 0, 512)

            attn_j0(0)
            att_mul(0, pa_att[0], pa_j0.pop(0), 0, 512)

            # ---------------- steady heads ----------------
            def emit_steady(h):
                prev = h - 1
                for i in (1, 2, 3):
                    sc(h, i)
                sc45(h)
                # attnv j0 of h: mms ready after e3; PE reaches them here
                attn_j0(h)
                sc(h + 1, 0)
                sc67(h)
                # j0 multiply of h (dep: bcast just emitted)
                att_mul(h, pa_att[h], pa_j0.pop(h), 0, 512)
                # attnv j1 of prev, reusing its accumulator (j0 mul done)
                pa_prev = pa_att.pop(prev)
                att_mms(prev, pa_prev, [0, 1, 2, 3], 512, 1024, True, False,
                        base=512)
                att_mms(prev, pa_prev, [4, 5, 6, 7], 512, 1024, False, True,
                        base=512)
                rbj1 = att_recip_bcast(pa_prev, 0, 512)
                att_mul(prev, pa_prev, rbj1, 512, 1024, p0=0)
                pts_tiles.pop(prev)

            for h in range(1, 7):
                emit_steady(h)

            # ---------------- head 7 + tail ----------------
            p7 = get_pts(7)
            pa6 = pa_att.pop(6)
            sc(7, 1)
            sc(7, 2)
            # j1 of head 6, interleaved with head-7 scores
            att_mms(6, pa6, [0, 1], 512, 1024, True, False, base=512)
            sc(7, 3)
            att_mms(6, pa6, [2, 3], 512, 1024, False, False, base=512)
            sc45(7)
            # attnv j0 of head 7, group A: cols [0:256) needs chunks 0,1
            pa7 = pap.tile([33, 512], F32, tag="pa", name="pa7")
            nc.tensor.matmul(pa7[:, 0:256], vaug[:, 0, 7, :], p7[:, 0:256],
                             start=True, stop=False)
            nc.tensor.matmul(pa7[:, 128:256], vaug[:, 1, 7, :],
                             p7[:, pcol(1, 128):pcol(1, 256)],
                             start=False, stop=True)
            rb_a = att_recip_bcast(pa7, 0, 256)
            att_mms(6, pa6, [4, 5, 6, 7], 512, 1024, False, True, base=512)
            rbj1_6 = att_recip_bcast(pa6, 0, 512)
            # group B: cols [256:512) needs chunks 0-3
            nc.tensor.matmul(pa7[:, 256:512], vaug[:, 0, 7, :], p7[:, 256:512],
                             start=True, stop=False)
            nc.tensor.matmul(pa7[:, 256:512], vaug[:, 1, 7, :],
                             p7[:, pcol(1, 256):pcol(1, 512)],
                             start=False, stop=False)
            nc.tensor.matmul(pa7[:, 256:512], vaug[:, 2, 7, :],
                             p7[:, pcol(2, 256):pcol(2, 512)],
                             start=False, stop=False)
            nc.tensor.matmul(pa7[:, 384:512], vaug[:, 3, 7, :],
                             p7[:, pcol(3, 384):pcol(3, 512)],
                             start=False, stop=True)
            rb_b = att_recip_bcast(pa7, 256, 512)
            sc67(7)
            pts_tiles.pop(6)
            # tail accumulators: [512:768) at cols [0:256) and [768:1024)
            # at cols [256:512) of one shared small-ring tile
            pa_cde = sml.tile([33, 512], F32, tag="sm", name="pa_cde")
            att_mms(7, pa_cde, [0, 1, 2, 3, 4, 5], 512, 768, True, True,
                    base=512)
            att_mms(7, pa_cde, [0, 1, 2, 3, 4, 5], 768, 1024, True, False,
                    base=512)
            # muls + outproj pieces, pipelined against the exp tail
            att_mul(7, pa7, rb_a, 0, 256)
            po_piece(0, 256, 'act')
            rb_c = att_recip_bcast(pa_cde, 0, 256)
            att_mul(6, pa6, rbj1_6, 512, 1024, p0=0)
            att_mul(7, pa7, rb_b, 256, 512, p0=256)
            po_piece(256, 512, 'act')
            att_mms(7, pa_cde, [6, 7], 768, 1024, False, True, base=512)
            rb_de = att_recip_bcast(pa_cde, 256, 512)
            att_mul(7, pa_cde, rb_c, 512, 768, p0=0)
            po_piece(512, 768, 'act')
            att_mul(7, pa_cde, rb_de, 768, 1024, p0=256)
            po_piece(768, 1024, 'act')

    nc.compile()
    return nc


def get_program():
    if "nc" not in _CACHE:
        _CACHE["nc"] = _build_program()
    return _CACHE["nc"]


def kernel(x, wq, bq, wkv, bkv, wproj, bproj):
    import ml_dtypes
    from concourse.bass_utils import run_bass_kernel_spmd

    nc = get_program()

    x = np.asarray(x, dtype=np.float32)
    n = x.shape[0]
    assert n == N_CORES and x.shape[1:] == (C, 32, 32)

    scale = 1.0 / np.sqrt(np.float32(D))
    wq_s = np.asarray(wq, np.float32) * scale
    bq_s = np.asarray(bq, np.float32) * scale
    wk = np.asarray(wkv[:E], np.float32)
    bk = np.asarray(bkv[:E], np.float32)
    wv = np.asarray(wkv[E:], np.float32)
    bv = np.asarray(bkv[E:], np.float32)
    wproj = np.asarray(wproj, np.float32)
    bproj_eff = (np.asarray(bproj, np.float32)
                 + wproj.astype(np.float64) @ bv.astype(np.float64)).astype(np.float32)

    # msk2: [identity | -1e30 * strict_lower(r > sq)]
    ident = np.eye(128, dtype=np.float32)
    mlow = np.where(np.arange(128)[:, None] > np.arange(128)[None, :],
                    np.float32(-1e30), np.float32(0.0))
    msk2 = np.concatenate([ident, mlow], axis=1).astype(ml_dtypes.bfloat16)

    shared = {
        "wqk0": np.ascontiguousarray(
            np.concatenate([wq_s.T[:, 0:128], wk.T[:, 0:128]], axis=1)),
        "wqk1": np.ascontiguousarray(
            np.concatenate([wq_s.T[:, 128:256], wk.T[:, 128:256]], axis=1)),
        "wvt": np.ascontiguousarray(wv.T),
        "wpt": np.ascontiguousarray(wproj.T.astype(ml_dtypes.bfloat16)),
        "biasd": np.ascontiguousarray(
            np.concatenate([bq_s, bk, bproj_eff])),
        "bprow": np.ascontiguousarray(
            bproj_eff.reshape(1, 256).astype(ml_dtypes.bfloat16)),
        "msk2": np.ascontiguousarray(msk2),
    }
    in_maps = [
        {"xin": np.ascontiguousarray(x[i].reshape(C, S)), **shared}
        for i in range(N_CORES)
    ]
    res = run_bass_kernel_spmd(nc, in_maps, core_ids=list(range(N_CORES)))
    out = np.stack([res.results[i]["out"].reshape(O, 32, 32) for i in range(N_CORES)])
    return out.astype(np.float32)
